# revision 37
# baseline (speedup 1.0000x reference)
"""Fused Fourier-block kernel for TRN2 (8 NeuronCores, data-parallel).

Reference computation (per token, C=1024, H=4096):
    h  = LN1(x)
    f  = real(FFT_C(h)) = h @ COS            (COS[n,k] = cos(2*pi*n*k/C))
    x2 = x + LNf(f)
    h2 = LN2(x2)
    m  = gelu_exact(h2 @ w1 + b1)
    out = x2 + m @ w2 + b2

Strategy: shard the 4*2048 = 8192 tokens over 8 cores (1024 tokens each).
All device math is done with activations CHANNEL-MAJOR ([channel, token]),
so every matmul consumes weights in their natural [in, out] layout and
chains without any device-side transposes (the host transposes each x shard
on the way in and the output shard on the way out).  LayerNorm reductions
over the channel (partition) dim are done on the TensorEngine as
ones-matmuls whose [128, T] PSUM output broadcasts the per-token sums to
every partition.

Precision/throughput: the DFT matmul stays fp16; half of each MLP
contraction (A8/B8 DoubleRow pairs) runs in fp8e4, which doubles PE
throughput for those chunks (weights pre-scaled by S1/S2 on host, the
inverse scale folded into the psum-evacuating activation; h2/gelu outputs
written in fp8 directly by DVE/ACT).  Sum-of-squares stats reductions also
run as fp8 DoubleRow (x^2 fits e4m3; its ~2.7% noise averages out over
C=1024 terms), and sum f^2 uses the real-FFT mirror symmetry to square
only half the spectrum.  Measured rel err 1.6e-2 vs the 2e-2 gate —
deterministic (fixed inputs), verified on hardware.
"""

from contextlib import ExitStack

import ml_dtypes
import numpy as np

import concourse.bacc as bacc
import concourse.mybir as mybir
import concourse.tile as tile
from concourse.bass_utils import run_bass_kernel_spmd

AF = mybir.ActivationFunctionType
ALU = mybir.AluOpType

P = 128          # SBUF partitions
C = 1024         # channel dim
H = 4096         # MLP hidden dim
KO = C // P      # 8 channel chunks
HO = H // P      # 32 hidden chunks
TOK = 1024       # tokens per core
TT = 512         # token tile (matmul moving dim)
NT = TOK // TT   # 2 token tiles per core
N_CORES = 8
EPS = 1e-5

F32 = mybir.dt.float32
F32R = mybir.dt.float32r
F16 = mybir.dt.float16
F8 = mybir.dt.float8e4
DRMODE = mybir.MatmulPerfMode.DoubleRow

# fp8 fraction of the MLP matmuls (DoubleRow pairs).  A8 of the KO//2=4
# mlp1 contraction chunk-pairs and B8 of the HO//2=16 mlp2 hidden
# block-pairs run in fp8e4; the rest stay fp16.  Weights are pre-scaled
# on host (S1/S2) to keep fp8 off the subnormal range; the inverse scale
# is folded into the psum-evacuating activation.
A8 = 2
B8 = 16
S1 = 16.0
S2 = 32.0
KF8 = 2 * A8          # fp8 channel chunks in mlp1
HF8 = 2 * B8          # fp8 hidden blocks in mlp2

# packed param columns (each [1024] vector becomes [128, 8] partition-major)
_PCOLS = {
    "ln1_g": 0, "ln1_b": 8, "lnf_g": 16, "lnf_b": 24,
    "ln2_g": 32, "ln2_b": 40, "b2": 48,
}
_B1_COL = 56  # b1 occupies cols 56..88
_GCS_COL = 88   # colsum(g*COS) for the 5 direct FFT chunks
_BFX_COL = 93   # (ln1_b @ COS) for the 5 direct FFT chunks
_G0_COL = 98    # ln1_g[0] replicated
_B0_COL = 99    # ln1_b[0] replicated
_PWIDTH = 100


def _build_nc(flags):
    nc = bacc.Bacc()

    xT16 = nc.declare_dram_parameter("xT16", [C, TOK], F16, isOutput=False)
    fcos = nc.declare_dram_parameter("fcos", [C, 5 * P], F16, isOutput=False)
    w1b8 = nc.declare_dram_parameter("w1b8", [HO, P, KF8, P], F8, isOutput=False)
    w1b = nc.declare_dram_parameter("w1b", [HO, P, KO - KF8, P], F16, isOutput=False)
    w2b8 = nc.declare_dram_parameter("w2b8", [KO, P, HF8, P], F8, isOutput=False)
    mir = nc.declare_dram_parameter("mir", [2, P, P], F16, isOutput=False)
    params = nc.declare_dram_parameter("params", [P, _PWIDTH], F32, isOutput=False)
    outT = nc.declare_dram_parameter("outT", [C, TOK], F16, isOutput=True)

    xT16_r = xT16.rearrange("(ko kp) t -> kp ko t", kp=P)
    fcos_r = fcos.rearrange("(ko kp) m -> kp ko m", kp=P)
    outT_r = outT.rearrange("(co cp) t -> cp co t", cp=P)

    with tile.TileContext(nc) as tc, ExitStack() as ctx:
        persist = ctx.enter_context(tc.tile_pool(name="persist", bufs=1))
        tmp = ctx.enter_context(tc.tile_pool(name="tmp", bufs=3))
        stat = ctx.enter_context(tc.tile_pool(name="stat", bufs=3))
        outp = ctx.enter_context(tc.tile_pool(name="outp", bufs=2))

        # ---------- constants ----------
        ones_h = persist.tile([P, P], F16)
        nc.vector.memset(ones_h, 1.0)
        ones_8 = persist.tile([P, 2, P], F8)
        nc.vector.memset(ones_8, 1.0)
        half_h = persist.tile([P, P], F16)
        nc.vector.memset(half_h, 0.5)
        eps_sb = persist.tile([P, 1], F32)
        nc.vector.memset(eps_sb, EPS)

        par_sb = persist.tile([P, _PWIDTH], F32)

        def pcol(name, k):
            c0 = _PCOLS[name] + k
            return par_sb[:, c0 : c0 + 1]

        # activations that live across both phases
        x2_sb = [persist.tile([P, KO, TT], F16, name=f"x2{t}") for t in range(NT)]
        h28_sb = [persist.tile([P, KF8, TT], F8, name=f"h28_{t}") for t in range(NT)]
        h2_sb = [
            persist.tile([P, KO - KF8, TT], F16, name=f"h2{t}") for t in range(NT)
        ]

        def ln_stats(src, ones, ps_s, ps_q, mu_bcast_src=None, sq_act=False):
            """src: [P, KO, TT] fp16 tile. Returns (mu16, rstd16) [P, TT] fp16
            broadcast across all partitions. If mu_bcast_src is given (a
            [1, TT] AP already equal to the mean), broadcast it with a single
            K=1 matmul instead of the 8-matmul sum reduction.  The sum-of-
            squares reduction runs as fp8 DoubleRow (x^2 fits e4m3 range and
            its 2.7% noise averages out over C=1024 terms); sq_act moves the
            squaring to the Scalar engine for windows where DVE is the
            critical chain."""
            psum_s = ps_s.tile([P, TT], F32, tag="ps_s")
            psum_q = ps_q.tile([P, TT], F32, tag="ps_q")
            if mu_bcast_src is not None:
                nc.tensor.matmul(
                    psum_s, lhsT=ones[0:1, :], rhs=mu_bcast_src,
                    start=True, stop=True,
                )
            else:
                for k in range(KO):
                    nc.tensor.matmul(
                        psum_s, lhsT=ones, rhs=src[:, k, :],
                        start=(k == 0), stop=(k == KO - 1),
                    )
            for j in range(KO // 2):
                sq8 = tmp.tile([P, 2, TT], F8, tag="sq8")
                for jj in range(2):
                    k = 2 * j + jj
                    if sq_act:
                        nc.scalar.activation(sq8[:, jj, :], src[:, k, :], AF.Square)
                    else:
                        nc.vector.tensor_mul(sq8[:, jj, :], src[:, k, :], src[:, k, :])
                nc.tensor.matmul(
                    psum_q, lhsT=ones_8, rhs=sq8,
                    start=(j == 0), stop=(j == KO // 2 - 1),
                    perf_mode=DRMODE,
                )
            mu_scale = 1.0 if mu_bcast_src is not None else 1.0 / C
            mu16 = stat.tile([P, TT], F16, tag="mu")
            nc.scalar.activation(mu16, psum_s, AF.Copy, scale=mu_scale)
            musq = stat.tile([P, TT], F32, tag="musq", bufs=4)
            nc.scalar.activation(musq, psum_s, AF.Square, scale=mu_scale)
            var = stat.tile([P, TT], F32, tag="var")
            nc.vector.scalar_tensor_tensor(
                var, psum_q, 1.0 / C, musq, ALU.mult, ALU.subtract
            )
            nc.scalar.activation(var, var, AF.Sqrt, bias=eps_sb)
            rstd = stat.tile([P, TT], F32, tag="rstd")
            nc.vector.reciprocal_approx_fast(rstd, var)
            rstd16 = stat.tile([P, TT], F16, tag="rstd16")
            nc.vector.tensor_copy(rstd16, rstd)
            return mu16, rstd16, musq

        def ln_apply_chunk(src, mu16, rstd16, gname, bname, dst, k):
            xc = tmp.tile([P, TT], F16, tag="xc")
            nc.vector.tensor_tensor(xc, src[:, k, :], mu16, ALU.subtract)
            nc.vector.tensor_tensor(xc, xc, rstd16, ALU.mult)
            nc.vector.tensor_scalar(
                dst[:, k, :], xc, pcol(gname, k), pcol(bname, k),
                ALU.mult, ALU.add,
            )

        def ln2_apply(t, mu16, rstd16, fp8_act=True, pool_subs=0):
            """h2 = LN2(x2); first KF8 chunks land in fp8.  With ln2_triv
            (gamma==1, beta==0 detected on host) the per-chunk chain is just
            sub + mult, the mult writing the fp8/fp16 tile directly.  The
            mean-subtractions run up front (mu is ready before rstd); the
            last `pool_subs` of them go to the otherwise-idle GpSimd engine.
            """
            ln2_triv = flags.get("ln2_triv", False)
            zs = []
            for k in range(KO):
                z = tmp.tile([P, TT], F16, tag="z", bufs=KO)
                eng = nc.gpsimd if k >= KO - pool_subs else nc.vector
                eng.tensor_tensor(z, x2_sb[t][:, k, :], mu16, ALU.subtract)
                zs.append(z)
            for k in range(KO):
                dst = (
                    h28_sb[t][:, k, :] if k < KF8
                    else h2_sb[t][:, k - KF8, :]
                )
                if ln2_triv:
                    nc.vector.tensor_tensor(dst, zs[k], rstd16, ALU.mult)
                elif k < KF8 and fp8_act:
                    xc = tmp.tile([P, TT], F16, tag="xc")
                    nc.vector.tensor_tensor(xc, zs[k], rstd16, ALU.mult)
                    nc.scalar.activation(
                        dst, xc, AF.Identity,
                        bias=pcol("ln2_b", k), scale=pcol("ln2_g", k),
                    )
                else:
                    nc.vector.tensor_tensor(zs[k], zs[k], rstd16, ALU.mult)
                    nc.vector.tensor_scalar(
                        dst, zs[k],
                        pcol("ln2_g", k), pcol("ln2_b", k),
                        ALU.mult, ALU.add,
                    )

        # ===== software pipeline across the two token tiles ================
        # PE-order: phase1(t0) | stats1(t1) | MLP1(t0) | FFT..LN2(t1) |
        # MLP2(t0) | MLP1(t1) | MLP2(t1).  Tile t1's DVE/ACT-bound LayerNorm
        # chains hide under tile t0's PE-bound MLP matmul stream.
        ps_s = ctx.enter_context(tc.tile_pool(name="ps_s", bufs=1, space="PSUM"))
        ps_q = ctx.enter_context(tc.tile_pool(name="ps_q", bufs=1, space="PSUM"))
        ps_fft = ctx.enter_context(tc.tile_pool(name="ps_fft", bufs=3, space="PSUM"))
        # mlp1's gelu-evac and mlp2's out-evac phases never overlap, so one
        # triple-buffered pool serves both (saves a bank for ps_fft).
        ps_mlp = ctx.enter_context(tc.tile_pool(name="ps_mlp", bufs=3, space="PSUM"))
        ps_out = ps_mlp
        wblk = ctx.enter_context(tc.tile_pool(name="wblk", bufs=1))

        cm_fcos = tc.tile_pool(name="p_fcos", bufs=1, side="right")
        p_fcos = cm_fcos.__enter__()
        cm_xhf = [tc.tile_pool(name=f"p_xhf{t}", bufs=1, side="right")
                  for t in range(NT)]
        # open xhf1 BEFORE xhf0 so the right-side stack pops LIFO:
        # xhf0 (after phase1 t0), then xhf1, then fcos.
        p_xhf = [None, None]
        p_xhf[1] = cm_xhf[1].__enter__()
        p_xhf[0] = cm_xhf[0].__enter__()
        cm_m = [tc.tile_pool(name=f"p_m{t}", bufs=1) for t in range(NT)]

        x16_sb = [p_xhf[t].tile([P, KO, TT], F16, name=f"x16_{t}") for t in range(NT)]
        f_sb = [p_xhf[t].tile([P, KO, TT], F16, name=f"f{t}") for t in range(NT)]
        fcos_sb = p_fcos.tile([P, KO, 5 * P], F16)
        mir_sb = persist.tile([P, 2, P], F16)
        m8_sb = [None, None]

        # tile-0 x16 only; everything else is emitted after the tile-0 stats
        # chain so those matmuls' DMA watermarks cover just these chunks.
        # Single-chunk transfers alternating across two issue queues halve
        # the arrival cadence the startup stats matmuls trickle behind.
        for k in range(KO):
            eng = nc.sync if k % 2 == 0 else nc.scalar
            eng.dma_start(x16_sb[0][:, k : k + 1, :], xT16_r[:, k : k + 1, 0:TT])
        nc.gpsimd.dma_start(par_sb, params[:, :])
        nc.gpsimd.dma_start(mir_sb, mir.rearrange("two q p -> q two p"))

        def load_fcos():
            for k in range(0, KO, 2):
                nc.sync.dma_start(fcos_sb[:, k : k + 2, :], fcos_r[:, k : k + 2, :])

        def load_x16_t1():
            # emitted late so the tile-0 stats/FFT waits get low DMA
            # watermarks; still issues early enough on the sync queue.
            for k in range(0, KO, 2):
                nc.sync.dma_start(
                    x16_sb[1][:, k : k + 2, :], xT16_r[:, k : k + 2, TT : 2 * TT]
                )

        def fft(t, mu16, rstd16, murstd16, mid=None):
            # raw = x16 @ (g*COS); f = rstd*raw - (mu*rstd)*gcs + bfx
            # (LN1 folded into the weights; matmuls depend only on x16).
            # mid[pair] callbacks emit the other tile's LN chain between
            # chunk-pairs so its DVE work starts as early as possible.
            # u_modes (host-detected): per m-chunk, 'none' means gcs==bfx==0
            # there, so the evac is a single rstd-multiply; 'row0' (the DC
            # column of a constant-gamma LN1) additionally patches partition 0
            # with two [1,TT] ops.  'full' keeps the general 3-op chain.
            u_modes = flags.get("u_modes", ("full",) * 5)
            for pair, ms in enumerate([(0, 1), (2, 3), (4,)]):
                if mid is not None and pair in mid:
                    mid[pair]()
                psums = [
                    ps_fft.tile([P, TT], F32, tag="fft", name=f"fft{j}")
                    for j in range(len(ms))
                ]
                for k in range(KO):
                    for j, m in enumerate(ms):
                        nc.tensor.matmul(
                            psums[j],
                            lhsT=fcos_sb[:, k, m * P : (m + 1) * P],
                            rhs=x16_sb[t][:, k, :],
                            start=(k == 0), stop=(k == KO - 1),
                        )
                for j, m in enumerate(ms):
                    if u_modes[m] == "full":
                        q1 = tmp.tile([P, TT], F16, tag="fq")
                        nc.vector.tensor_tensor(q1, psums[j], rstd16, ALU.mult)
                        u = tmp.tile([P, TT], F16, tag="fu")
                        nc.vector.tensor_scalar(
                            u, murstd16,
                            par_sb[:, _GCS_COL + m : _GCS_COL + m + 1],
                            par_sb[:, _BFX_COL + m : _BFX_COL + m + 1],
                            ALU.mult, ALU.subtract,
                        )
                        nc.vector.tensor_tensor(
                            f_sb[t][:, m, :], q1, u, ALU.subtract
                        )
                        continue
                    nc.vector.tensor_tensor(
                        f_sb[t][:, m, :], psums[j], rstd16, ALU.mult
                    )
                    if u_modes[m] == "row0":
                        u0 = tmp.tile([P, TT], F16, tag="fu")
                        nc.vector.tensor_scalar(
                            u0[0:1, :], murstd16[0:1, :],
                            par_sb[0:1, _GCS_COL + m : _GCS_COL + m + 1],
                            par_sb[0:1, _BFX_COL + m : _BFX_COL + m + 1],
                            ALU.mult, ALU.subtract,
                        )
                        nc.vector.tensor_tensor(
                            f_sb[t][0:1, m, :], f_sb[t][0:1, m, :],
                            u0[0:1, :], ALU.subtract,
                        )
            for m in (5, 6, 7):
                psum_m_ = ps_fft.tile([P, TT], F32, tag="fft", name="fftm")
                nc.tensor.matmul(
                    psum_m_, lhsT=mir_sb[:, 0, :], rhs=f_sb[t][:, 7 - m, :],
                    start=True, stop=False,
                )
                nc.tensor.matmul(
                    psum_m_, lhsT=mir_sb[:, 1, :], rhs=f_sb[t][:, 8 - m, :],
                    start=False, stop=True,
                )
                nc.scalar.activation(f_sb[t][:, m, :], psum_m_, AF.Copy)

        def lnf_stats(t, mu16, rstd16, sq_act=False):
            """stats of f: mean(f) == LN1(x)[0] == g0*(x0-mu)*rstd + b0.
            real-FFT symmetry: f[c] == f[C-c], so sum_c f^2 =
            2*sum_{c<512} f^2 - f0^2 + f512^2 — only chunks 0..3 get
            squared; the two single-row corrections ride a K=1 matmul
            with a 0.5 lhsT (the final ACT scale is 2/C)."""
            psum_s = ps_s.tile([P, TT], F32, tag="ps_s")
            psum_q = ps_q.tile([P, TT], F32, tag="ps_q")
            nc.tensor.matmul(
                psum_s, lhsT=ones_h[0:1, :], rhs=x16_sb[t][0:1, 0, :],
                start=True, stop=True,
            )
            sq_eng = nc.scalar if sq_act else nc.vector
            for k in range(KO // 2):
                sq = tmp.tile([P, TT], F16, tag="sq")
                if sq_act:
                    nc.scalar.activation(sq, f_sb[t][:, k, :], AF.Square)
                else:
                    nc.vector.tensor_mul(sq, f_sb[t][:, k, :], f_sb[t][:, k, :])
                nc.tensor.matmul(
                    psum_q, lhsT=ones_h, rhs=sq,
                    start=(k == 0), stop=False,
                )
            corr = tmp.tile([P, TT], F16, tag="corr")
            c2 = tmp.tile([P, TT], F16, tag="corr2")
            if sq_act:
                nc.scalar.activation(corr[0:1, :], f_sb[t][0:1, 4, :], AF.Square)
                nc.scalar.activation(c2[0:1, :], f_sb[t][0:1, 0, :], AF.Square)
            else:
                nc.vector.tensor_mul(
                    corr[0:1, :], f_sb[t][0:1, 4, :], f_sb[t][0:1, 4, :]
                )
                nc.vector.tensor_mul(
                    c2[0:1, :], f_sb[t][0:1, 0, :], f_sb[t][0:1, 0, :]
                )
            nc.vector.tensor_tensor(corr[0:1, :], corr[0:1, :], c2[0:1, :],
                                    ALU.subtract)
            nc.tensor.matmul(
                psum_q, lhsT=half_h[0:1, :], rhs=corr[0:1, :],
                start=False, stop=True,
            )
            s1 = stat.tile([P, TT], F16, tag="mu")  # becomes muf16
            nc.vector.tensor_tensor(s1, psum_s, mu16, ALU.subtract)
            nc.vector.tensor_tensor(s1, s1, rstd16, ALU.mult)
            if not flags.get("s1_triv", False):
                nc.vector.tensor_scalar(
                    s1, s1,
                    par_sb[:, _G0_COL : _G0_COL + 1],
                    par_sb[:, _B0_COL : _B0_COL + 1],
                    ALU.mult, ALU.add,
                )
            musq = stat.tile([P, TT], F32, tag="musq", bufs=4)
            nc.scalar.activation(musq, s1, AF.Square)
            var = stat.tile([P, TT], F32, tag="var")
            nc.vector.scalar_tensor_tensor(
                var, psum_q, 2.0 / C, musq, ALU.mult, ALU.subtract
            )
            nc.scalar.activation(var, var, AF.Sqrt, bias=eps_sb)
            rstd = stat.tile([P, TT], F32, tag="rstd")
            nc.vector.reciprocal_approx_fast(rstd, var)
            rstdf16 = stat.tile([P, TT], F16, tag="rstd16")
            nc.vector.tensor_copy(rstdf16, rstd)
            return s1, rstdf16

        def lnf_residual(t, muf16, rstdf16, pool_subs=0):
            # the mean-subtraction depends only on muf (ready well before
            # rstdf) — run all 8 up front so only 2 ops/chunk trail rstdf;
            # the last `pool_subs` subtractions go to the idle GpSimd engine.
            fns = []
            for k in range(KO):
                fn = tmp.tile([P, TT], F16, tag="fn", bufs=KO)
                eng = nc.gpsimd if k >= KO - pool_subs else nc.vector
                eng.tensor_tensor(fn, f_sb[t][:, k, :], muf16, ALU.subtract)
                fns.append(fn)
            for k in range(KO):
                nc.vector.tensor_tensor(fns[k], fns[k], rstdf16, ALU.mult)
                nc.vector.affine_then_add(
                    x2_sb[t][:, k, :], fns[k], x16_sb[t][:, k, :],
                    pcol("lnf_g", k), pcol("lnf_b", k),
                )


        def res_pre(t, muf16, rstdf16, mu2_16):
            """lnf-residual, LN2 mean-subtract and the x2 stats squares
            (lnf_triv fast path: mean(LNf(f)) == 0 exactly, so mu2 is tile
            t's mu1 and no x2 sum reduction is needed).  The x2 chunk chain
            is kept to 2 DVE ops/chunk (fn-subs all run up-front; stats
            squares ride ACT; the LN2 subtractions are deferred, with
            chunks 0-1 interleaved early so mlp1's first DoubleRow pair is
            ready as soon as rstd2 lands).  Returns (zs, sq8_tiles)."""
            fns = []
            for k in range(KO):
                fn = tmp.tile([P, TT], F16, tag="fn", bufs=KO)
                nc.vector.tensor_tensor(fn, f_sb[t][:, k, :], muf16, ALU.subtract)
                fns.append(fn)
            zs, sq8s = [None] * KO, []
            for k in range(KO):
                nc.vector.tensor_tensor(fns[k], fns[k], rstdf16, ALU.mult)
                nc.vector.affine_then_add(
                    x2_sb[t][:, k, :], fns[k], x16_sb[t][:, k, :],
                    pcol("lnf_g", k), pcol("lnf_b", k),
                )
                if k % 2 == 0:
                    sq8 = tmp.tile(
                        [P, 2, TT], F8, tag="sq8x", bufs=4, name="sq8x"
                    )
                    sq8s.append(sq8)
                nc.scalar.activation(
                    sq8s[-1][:, k % 2, :], x2_sb[t][:, k, :], AF.Square
                )
                if k == 1:
                    for kk in (0, 1):
                        z = tmp.tile([P, TT], F16, tag="z", bufs=KO)
                        nc.vector.tensor_tensor(
                            z, x2_sb[t][:, kk, :], mu2_16, ALU.subtract
                        )
                        zs[kk] = z
            for k in range(2, KO):
                z = tmp.tile([P, TT], F16, tag="z", bufs=KO)
                nc.vector.tensor_tensor(
                    z, x2_sb[t][:, k, :], mu2_16, ALU.subtract
                )
                zs[k] = z
            return zs, sq8s

        def st2_var(sq8s, musq):
            psum_q = ps_q.tile([P, TT], F32, tag="ps_q")
            for j, sq8 in enumerate(sq8s):
                nc.tensor.matmul(
                    psum_q, lhsT=ones_8, rhs=sq8,
                    start=(j == 0), stop=(j == len(sq8s) - 1),
                    perf_mode=DRMODE,
                )
            var = stat.tile([P, TT], F32, tag="var")
            nc.vector.scalar_tensor_tensor(
                var, psum_q, 1.0 / C, musq, ALU.mult, ALU.subtract
            )
            nc.scalar.activation(var, var, AF.Sqrt, bias=eps_sb)
            rstd = stat.tile([P, TT], F32, tag="rstd")
            nc.vector.reciprocal_approx_fast(rstd, var)
            rstd16 = stat.tile([P, TT], F16, tag="rstd16")
            nc.vector.tensor_copy(rstd16, rstd)
            return rstd16

        def ln2_mults(t, zs, rstd16):
            for k in range(KO):
                dst = (
                    h28_sb[t][:, k, :] if k < KF8
                    else h2_sb[t][:, k - KF8, :]
                )
                if flags.get("ln2_triv", False):
                    nc.vector.tensor_tensor(dst, zs[k], rstd16, ALU.mult)
                else:
                    nc.vector.tensor_tensor(zs[k], zs[k], rstd16, ALU.mult)
                    nc.vector.tensor_scalar(
                        dst, zs[k],
                        pcol("ln2_g", k), pcol("ln2_b", k),
                        ALU.mult, ALU.add,
                    )

        w1_pend = {}
        w2_pend = {}

        def issue_w1(h):
            w1blk8 = wblk.tile([P, KF8, P], F8, tag="w1blk8", bufs=8)
            nc.sync.dma_start(w1blk8, w1b8[h])
            w1blk = wblk.tile([P, KO - KF8, P], F16, tag="w1blk", bufs=8)
            nc.sync.dma_start(w1blk, w1b[h])
            w1_pend[h] = (w1blk8, w1blk)

        def issue_w2(c):
            w2blk8 = wblk.tile([P, HF8, P], F8, tag="w2blk8", bufs=3)
            nc.sync.dma_start(w2blk8, w2b8[c])
            w2_pend[c] = w2blk8

        def mlp1(t, h_order=None, keep_last=0, mid=None):
            for bi, h in enumerate(h_order if h_order is not None else range(HO)):
                if mid is not None and bi in mid:
                    mid[bi]()
                if h in w1_pend:
                    w1blk8, w1blk = w1_pend.pop(h)
                else:
                    issue_w1(h)
                    w1blk8, w1blk = w1_pend.pop(h)
                psum_m = ps_mlp.tile([P, TT], F32, tag="mlp1")
                for i in range(A8):
                    nc.tensor.matmul(
                        psum_m,
                        lhsT=w1blk8[:, 2 * i : 2 * i + 2, :],
                        rhs=h28_sb[t][:, 2 * i : 2 * i + 2, :],
                        start=(i == 0), stop=False,
                        perf_mode=DRMODE,
                    )
                for k in range(KO - KF8):
                    nc.tensor.matmul(
                        psum_m, lhsT=w1blk[:, k, :], rhs=h2_sb[t][:, k, :],
                        start=False, stop=(k == KO - KF8 - 1),
                    )
                bias = par_sb[:, _B1_COL + h : _B1_COL + h + 1]
                nc.scalar.activation(
                    m8_sb[t][:, h, :], psum_m, AF.Gelu,
                    bias=bias, scale=1.0 / S1,
                )
                if keep_last and h >= HO - keep_last:
                    # leave the block registered for the next tile's reuse
                    w1_pend[h] = (w1blk8, w1blk)

        def mlp2(t, c_order=None, keep_last=0):
            for c in (c_order if c_order is not None else range(KO)):
                if c in w2_pend:
                    w2blk8 = w2_pend.pop(c)
                else:
                    issue_w2(c)
                    w2blk8 = w2_pend.pop(c)
                psum_o = ps_out.tile([P, TT], F32, tag="mlp1")
                # residual pre-loaded into the psum (scaled to match the
                # S2-scaled weights); all matmuls accumulate on top, and the
                # evacuating activation divides back and adds b2 — no
                # separate DVE residual-add on the output path.
                nc.vector.tensor_scalar_mul(psum_o, x2_sb[t][:, c, :], S2)
                for i in range(B8):
                    nc.tensor.matmul(
                        psum_o,
                        lhsT=w2blk8[:, 2 * i : 2 * i + 2, :],
                        rhs=m8_sb[t][:, 2 * i : 2 * i + 2, :],
                        start=False, stop=(i == B8 - 1),
                        perf_mode=DRMODE,
                    )
                ob = outp.tile([P, TT], F16, tag="ob")
                nc.scalar.activation(
                    ob, psum_o, AF.Identity, bias=pcol("b2", c), scale=1.0 / S2
                )
                nc.sync.dma_start(outT_r[:, c, t * TT : (t + 1) * TT], ob)
                if keep_last and c >= KO - keep_last:
                    w2_pend[c] = w2blk8

        # ---- tile 0 phase 1, tile-1 work woven in as PE filler ----
        lnf_triv = flags.get("lnf_triv", False)
        st1_0 = ln_stats(x16_sb[0], ones_h, ps_s, ps_q)
        load_fcos()
        mrs0 = stat.tile([P, TT], F16, tag="mrs", name="mrs0", bufs=2)
        nc.vector.tensor_tensor(mrs0, st1_0[0], st1_0[1], ALU.mult)
        fft(0, *st1_0[:2], mrs0)
        load_x16_t1()
        st1_1 = ln_stats(x16_sb[1], ones_h, ps_s, ps_q, sq_act=True)
        mrs1 = stat.tile([P, TT], F16, tag="mrs", name="mrs1", bufs=2)
        nc.vector.tensor_tensor(mrs1, st1_1[0], st1_1[1], ALU.mult)

        stf0 = [None]
        zsq0 = [None]

        def _mid1():
            stf0[0] = lnf_stats(0, *st1_0[:2], sq_act=True)

        if lnf_triv:
            # res_pre woven after the THIRD pair-group's matmuls so tile-1's
            # fft psum evacuations keep DVE-queue priority over the 24-op
            # residual chain (mirror matmuls would stall behind it otherwise).
            def _mid2():
                zsq0[0] = res_pre(0, stf0[0][0], stf0[0][1], st1_0[0])

            fft(1, *st1_1[:2], mrs1, mid={0: _mid1, 1: _mid2})
            rstd2_0 = st2_var(zsq0[0][1], st1_0[2])
            ln2_mults(0, zsq0[0][0], rstd2_0)
        else:
            def _mid2g():
                lnf_residual(0, *stf0[0], pool_subs=4)

            fft(1, *st1_1[:2], mrs1, mid={0: _mid1, 1: _mid2g})
            st2_0 = ln_stats(x2_sb[0], ones_h, ps_s, ps_q, sq_act=True)
            ln2_apply(0, *st2_0[:2], pool_subs=4)

        # ---- pipeline ----
        cm_xhf[0].__exit__(None, None, None)
        p_m0 = cm_m[0].__enter__()
        m8_sb[0] = p_m0.tile([P, HF8, TT], F8, name="m8_0")

        # tile-1's LN chain is woven INTO mlp1(0)'s matmul stream (not after
        # it): its PE bits execute between h-blocks and its serial DVE/ACT
        # chain drains ~20us earlier, so the (now much shorter, all-fp8)
        # mlp2(0) stream no longer ends before h28_1 is ready.
        st2_1 = [None]
        stf1 = [None]
        zsq1 = [None]
        rstd2_1 = [None]

        def _m1a():
            stf1[0] = lnf_stats(1, *st1_1[:2])

        if lnf_triv:
            def _m1b():
                zsq1[0] = res_pre(1, stf1[0][0], stf1[0][1], st1_1[0])

            def _m1c():
                rstd2_1[0] = st2_var(zsq1[0][1], st1_1[2])

            def _m1d():
                ln2_mults(1, zsq1[0][0], rstd2_1[0])
        else:
            def _m1b():
                lnf_residual(1, *stf1[0], pool_subs=4)

            def _m1c():
                st2_1[0] = ln_stats(x2_sb[1], ones_h, ps_s, ps_q)

            def _m1d():
                ln2_apply(1, *st2_1[0][:2], fp8_act=False, pool_subs=4)

        mlp1(0, keep_last=8, mid={5: _m1a, 9: _m1b, 16: _m1c, 20: _m1d})

        cm_xhf[1].__exit__(None, None, None)
        cm_fcos.__exit__(None, None, None)
        p_m1 = cm_m[1].__enter__()
        m8_sb[1] = p_m1.tile([P, HF8, TT], F8, name="m8_1")

        mlp2(0, keep_last=3)
        mlp1(1, h_order=list(range(HO - 8, HO)) + list(range(HO - 8)))
        mlp2(1, c_order=[5, 6, 7, 0, 1, 2, 3, 4])

        cm_m[1].__exit__(None, None, None)
        cm_m[0].__exit__(None, None, None)

    nc.compile()
    return nc


_NC_CACHE: dict = {}


def _get_nc(flags):
    key = (
        flags["u_modes"], flags["ln2_triv"], flags["s1_triv"],
        flags["lnf_triv"],
    )
    if key not in _NC_CACHE:
        _NC_CACHE[key] = _build_nc(flags)
    return _NC_CACHE[key]


def _host_flags(inputs):
    """Detect trivial LN params so the build can drop dead device work.
    The general path is kept for any input where these don't hold."""
    g1 = np.asarray(inputs["ln1_g"], np.float64)
    b1v = np.asarray(inputs["ln1_b"], np.float64)
    n = np.arange(C, dtype=np.float64)
    cosm = np.cos((np.outer(n, n[: 5 * P]) % C) * (2.0 * np.pi / C))
    gcs = (g1[:, None] * cosm).sum(axis=0)
    bfx = (b1v[:, None] * cosm).sum(axis=0)
    mask = (np.abs(gcs) > 1e-6) | (np.abs(bfx) > 1e-6)
    u_modes = []
    for m in range(5):
        mm = mask[m * P : (m + 1) * P]
        if not mm.any():
            u_modes.append("none")
        elif m == 0 and mm[0] and not mm[1:].any():
            u_modes.append("row0")
        else:
            u_modes.append("full")
    ln2_triv = bool(
        np.allclose(inputs["ln2_g"], 1.0) and np.allclose(inputs["ln2_b"], 0.0)
    )
    s1_triv = bool(abs(g1[0] - 1.0) < 1e-12 and abs(b1v[0]) < 1e-12)
    lg = np.asarray(inputs["lnf_g"], np.float64)
    lb = np.asarray(inputs["lnf_b"], np.float64)
    # constant lnf gain + zero lnf bias make mean(LNf(f)) vanish exactly
    # (sum_c f[c] = C*h[0] = C*mean(f)), so mu2 == mu1 per token.
    lnf_triv = bool(np.ptp(lg) < 1e-12 and np.allclose(lb, 0.0))
    return {
        "u_modes": tuple(u_modes), "ln2_triv": ln2_triv,
        "s1_triv": s1_triv, "lnf_triv": lnf_triv,
    }


def _pack_params(inputs):
    p = np.zeros((P, _PWIDTH), np.float32)
    for name, col in _PCOLS.items():
        p[:, col : col + 8] = np.asarray(inputs[name], np.float32).reshape(8, P).T
    p[:, _B1_COL : _B1_COL + HO] = (
        np.asarray(inputs["b1"], np.float32).reshape(HO, P).T
    )
    n = np.arange(C, dtype=np.float64)
    cosm = np.cos((np.outer(n, n[: 5 * P]) % C) * (2.0 * np.pi / C))
    g1 = np.asarray(inputs["ln1_g"], np.float64)
    b1v = np.asarray(inputs["ln1_b"], np.float64)
    gcs = (g1[:, None] * cosm).sum(axis=0)          # [640]
    bfx = (b1v[:, None] * cosm).sum(axis=0)         # [640]
    p[:, _GCS_COL : _GCS_COL + 5] = gcs.reshape(5, P).T
    p[:, _BFX_COL : _BFX_COL + 5] = bfx.reshape(5, P).T
    p[:, _G0_COL] = np.float32(g1[0])
    p[:, _B0_COL] = np.float32(b1v[0])
    return p


def _run(inputs, trace=False):
    x = np.asarray(inputs["x"], np.float32)
    B, N, Cc = x.shape
    assert (B * N, Cc) == (N_CORES * TOK, C)
    x2d = x.reshape(B * N, C)

    n = np.arange(C, dtype=np.float64)
    # only the first 5*P output columns are computed directly (f[k] = f[C-k]);
    # LN1's per-channel gain is folded into the DFT matrix, its bias into a
    # per-output-channel additive term (see _pack_params).
    cosm = np.cos((np.outer(n, n[: 5 * P]) % C) * (2.0 * np.pi / C))
    g1 = np.asarray(inputs["ln1_g"], np.float64)
    fcos = (g1[:, None] * cosm).astype(np.float16)

    # weights pre-scaled by S1/S2 (undone in the psum-evacuating activation)
    # in block-contiguous layouts so each SBUF weight block is one clean DMA:
    # w1b[h, kp, ko, hc] = w1[ko*P+kp, h*P+hc]; w2b[c, hp, ho, cc] = w2[ho*P+hp, c*P+cc]
    w1s = np.asarray(inputs["w1"], np.float32) * S1
    w2s = np.asarray(inputs["w2"], np.float32) * S2
    w1bl_f = w1s.reshape(KO, P, HO, P).transpose(2, 1, 0, 3)
    w2bl_f = w2s.reshape(HO, P, KO, P).transpose(2, 1, 0, 3)
    w1bl8 = w1bl_f[:, :, :KF8, :].astype(ml_dtypes.float8_e4m3)
    w1bl = w1bl_f[:, :, KF8:, :].astype(np.float16)
    w2bl8 = w2bl_f[:, :, :HF8, :].astype(ml_dtypes.float8_e4m3)
    # mirror matrices: out[p,t] = f7m[P-p, t] (p>=1);  out[0,t] = f8m[0, t]
    mirm = np.zeros((2, P, P), np.float16)
    for p_ in range(1, P):
        mirm[0, P - p_, p_] = 1.0
    mirm[1, 0, 0] = 1.0
    params = _pack_params(inputs)

    in_maps = []
    for i in range(N_CORES):
        shard = x2d[i * TOK : (i + 1) * TOK, :]
        in_maps.append(
            {
                "xT16": np.ascontiguousarray(shard.T).astype(np.float16),
                "fcos": fcos,
                "w1b8": w1bl8,
                "w1b": w1bl,
                "w2b8": w2bl8,
                "mir": mirm,
                "params": params,
            }
        )

    nc = _get_nc(_host_flags(inputs))
    res = run_bass_kernel_spmd(nc, in_maps, core_ids=list(range(N_CORES)), trace=trace)

    out2d = np.empty((B * N, C), np.float32)
    for i in range(N_CORES):
        out2d[i * TOK : (i + 1) * TOK, :] = res.results[i]["outT"].T
    return out2d.reshape(B, N, C), res


def kernel(**inputs) -> np.ndarray:
    return _run(inputs)[0]



# revision 46
# speedup vs baseline: 1.0187x; 1.0187x over previous
"""Fused Fourier-block kernel for TRN2 (8 NeuronCores, data-parallel).

Reference computation (per token, C=1024, H=4096):
    h  = LN1(x)
    f  = real(FFT_C(h)) = h @ COS            (COS[n,k] = cos(2*pi*n*k/C))
    x2 = x + LNf(f)
    h2 = LN2(x2)
    m  = gelu_exact(h2 @ w1 + b1)
    out = x2 + m @ w2 + b2

Strategy: shard the 4*2048 = 8192 tokens over 8 cores (1024 tokens each).
All device math is done with activations CHANNEL-MAJOR ([channel, token]),
so every matmul consumes weights in their natural [in, out] layout and
chains without any device-side transposes (the host transposes each x shard
on the way in and the output shard on the way out).  LayerNorm reductions
over the channel (partition) dim are done on the TensorEngine as
ones-matmuls whose [128, T] PSUM output broadcasts the per-token sums to
every partition.

Precision/throughput: the DFT matmul stays fp16; half of each MLP
contraction (A8/B8 DoubleRow pairs) runs in fp8e4, which doubles PE
throughput for those chunks (weights pre-scaled by S1/S2 on host, the
inverse scale folded into the psum-evacuating activation; h2/gelu outputs
written in fp8 directly by DVE/ACT).  Sum-of-squares stats reductions also
run as fp8 DoubleRow (x^2 fits e4m3; its ~2.7% noise averages out over
C=1024 terms), and sum f^2 uses the real-FFT mirror symmetry to square
only half the spectrum.  Measured rel err 1.6e-2 vs the 2e-2 gate —
deterministic (fixed inputs), verified on hardware.
"""

from contextlib import ExitStack

import ml_dtypes
import numpy as np

import concourse.bacc as bacc
import concourse.mybir as mybir
import concourse.tile as tile
from concourse.bass_utils import run_bass_kernel_spmd

AF = mybir.ActivationFunctionType
ALU = mybir.AluOpType

P = 128          # SBUF partitions
C = 1024         # channel dim
H = 4096         # MLP hidden dim
KO = C // P      # 8 channel chunks
HO = H // P      # 32 hidden chunks
TOK = 1024       # tokens per core
TT = 512         # token tile (matmul moving dim)
NT = TOK // TT   # 2 token tiles per core
N_CORES = 8
EPS = 1e-5

F32 = mybir.dt.float32
F32R = mybir.dt.float32r
F16 = mybir.dt.float16
F8 = mybir.dt.float8e4
DRMODE = mybir.MatmulPerfMode.DoubleRow

# fp8 fraction of the MLP matmuls (DoubleRow pairs).  A8 of the KO//2=4
# mlp1 contraction chunk-pairs and B8 of the HO//2=16 mlp2 hidden
# block-pairs run in fp8e4; the rest stay fp16.  Weights are pre-scaled
# on host (S1/S2) to keep fp8 off the subnormal range; the inverse scale
# is folded into the psum-evacuating activation.
A8 = 2
B8 = 16
S1 = 16.0
S2 = 32.0
KF8 = 2 * A8          # fp8 channel chunks in mlp1
HF8 = 2 * B8          # fp8 hidden blocks in mlp2

# packed param columns (each [1024] vector becomes [128, 8] partition-major)
_PCOLS = {
    "ln1_g": 0, "ln1_b": 8, "lnf_g": 16, "lnf_b": 24,
    "ln2_g": 32, "ln2_b": 40, "b2": 48,
}
_B1_COL = 56  # b1 occupies cols 56..88
_GCS_COL = 88   # colsum(g*COS) for the 5 direct FFT chunks
_BFX_COL = 93   # (ln1_b @ COS) for the 5 direct FFT chunks
_G0_COL = 98    # ln1_g[0] replicated
_B0_COL = 99    # ln1_b[0] replicated
_PWIDTH = 100


def _build_nc(flags):
    nc = bacc.Bacc()

    xT16 = nc.declare_dram_parameter("xT16", [C, TOK], F16, isOutput=False)
    fcos = nc.declare_dram_parameter("fcos", [C, 5 * P], F16, isOutput=False)
    w1b8 = nc.declare_dram_parameter("w1b8", [HO, P, KF8, P], F8, isOutput=False)
    w1b = nc.declare_dram_parameter("w1b", [HO, P, KO - KF8, P], F16, isOutput=False)
    w2b8 = nc.declare_dram_parameter("w2b8", [KO, P, HF8, P], F8, isOutput=False)
    mir = nc.declare_dram_parameter("mir", [2, P, P], F16, isOutput=False)
    params = nc.declare_dram_parameter("params", [P, _PWIDTH], F32, isOutput=False)
    outT = nc.declare_dram_parameter("outT", [C, TOK], F16, isOutput=True)

    xT16_r = xT16.rearrange("(ko kp) t -> kp ko t", kp=P)
    fcos_r = fcos.rearrange("(ko kp) m -> kp ko m", kp=P)
    outT_r = outT.rearrange("(co cp) t -> cp co t", cp=P)

    with tile.TileContext(nc) as tc, ExitStack() as ctx:
        persist = ctx.enter_context(tc.tile_pool(name="persist", bufs=1))
        tmp = ctx.enter_context(tc.tile_pool(name="tmp", bufs=3))
        stat = ctx.enter_context(tc.tile_pool(name="stat", bufs=3))
        outp = ctx.enter_context(tc.tile_pool(name="outp", bufs=2))

        # ---------- constants ----------
        ones_h = persist.tile([P, P], F16)
        nc.vector.memset(ones_h, 1.0)
        ones_8 = persist.tile([P, 2, P], F8)
        nc.vector.memset(ones_8, 1.0)
        half_h = persist.tile([P, P], F16)
        nc.vector.memset(half_h, 0.5)
        eps_sb = persist.tile([P, 1], F32)
        nc.vector.memset(eps_sb, EPS)

        par_sb = persist.tile([P, _PWIDTH], F32)

        def pcol(name, k):
            c0 = _PCOLS[name] + k
            return par_sb[:, c0 : c0 + 1]

        # activations that live across both phases
        x2_sb = [persist.tile([P, KO, TT], F16, name=f"x2{t}") for t in range(NT)]
        h28_sb = [persist.tile([P, KF8, TT], F8, name=f"h28_{t}") for t in range(NT)]
        h2_sb = [
            persist.tile([P, KO - KF8, TT], F16, name=f"h2{t}") for t in range(NT)
        ]

        def ln_stats(src, ones, ps_s, ps_q, mu_bcast_src=None, sq_act=False,
                     sq_eng=None):
            """src: [P, KO, TT] fp16 tile. Returns (mu16, rstd16) [P, TT] fp16
            broadcast across all partitions. If mu_bcast_src is given (a
            [1, TT] AP already equal to the mean), broadcast it with a single
            K=1 matmul instead of the 8-matmul sum reduction.  The sum-of-
            squares reduction runs as fp8 DoubleRow (x^2 fits e4m3 range and
            its 2.7% noise averages out over C=1024 terms); sq_act moves the
            squaring to the Scalar engine for windows where DVE is the
            critical chain."""
            psum_s = ps_s.tile([P, TT], F32, tag="ps_s")
            psum_q = ps_q.tile([P, TT], F32, tag="ps_q")
            if mu_bcast_src is not None:
                nc.tensor.matmul(
                    psum_s, lhsT=ones[0:1, :], rhs=mu_bcast_src,
                    start=True, stop=True,
                )
            else:
                for k in range(KO):
                    nc.tensor.matmul(
                        psum_s, lhsT=ones, rhs=src[:, k, :],
                        start=(k == 0), stop=(k == KO - 1),
                    )
            if sq_eng is None:
                sq_eng = "act" if sq_act else "dve"
            for j in range(KO // 2):
                sq8 = tmp.tile([P, 2, TT], F8, tag="sq8")
                for jj in range(2):
                    k = 2 * j + jj
                    if sq_eng == "act":
                        nc.scalar.activation(sq8[:, jj, :], src[:, k, :], AF.Square)
                    elif sq_eng == "pool":
                        # Pool is slow (~1.3us/op) but idle during the fill;
                        # tile-1's stats squares have a loose deadline.
                        nc.gpsimd.tensor_mul(
                            sq8[:, jj, :], src[:, k, :], src[:, k, :]
                        )
                    else:
                        nc.vector.tensor_mul(sq8[:, jj, :], src[:, k, :], src[:, k, :])
                nc.tensor.matmul(
                    psum_q, lhsT=ones_8, rhs=sq8,
                    start=(j == 0), stop=(j == KO // 2 - 1),
                    perf_mode=DRMODE,
                )
            mu_scale = 1.0 if mu_bcast_src is not None else 1.0 / C
            mu16 = stat.tile([P, TT], F16, tag="mu")
            nc.scalar.activation(mu16, psum_s, AF.Copy, scale=mu_scale)
            musq = stat.tile([P, TT], F32, tag="musq", bufs=4)
            nc.scalar.activation(musq, psum_s, AF.Square, scale=mu_scale)
            var = stat.tile([P, TT], F32, tag="var")
            nc.vector.scalar_tensor_tensor(
                var, psum_q, 1.0 / C, musq, ALU.mult, ALU.subtract
            )
            nc.scalar.activation(var, var, AF.Sqrt, bias=eps_sb)
            rstd = stat.tile([P, TT], F32, tag="rstd")
            nc.vector.reciprocal_approx_fast(rstd, var)
            rstd16 = stat.tile([P, TT], F16, tag="rstd16")
            nc.vector.tensor_copy(rstd16, rstd)
            return mu16, rstd16, musq

        def ln_apply_chunk(src, mu16, rstd16, gname, bname, dst, k):
            xc = tmp.tile([P, TT], F16, tag="xc")
            nc.vector.tensor_tensor(xc, src[:, k, :], mu16, ALU.subtract)
            nc.vector.tensor_tensor(xc, xc, rstd16, ALU.mult)
            nc.vector.tensor_scalar(
                dst[:, k, :], xc, pcol(gname, k), pcol(bname, k),
                ALU.mult, ALU.add,
            )

        def ln2_apply(t, mu16, rstd16, fp8_act=True, pool_subs=0):
            """h2 = LN2(x2); first KF8 chunks land in fp8.  With ln2_triv
            (gamma==1, beta==0 detected on host) the per-chunk chain is just
            sub + mult, the mult writing the fp8/fp16 tile directly.  The
            mean-subtractions run up front (mu is ready before rstd); the
            last `pool_subs` of them go to the otherwise-idle GpSimd engine.
            """
            ln2_triv = flags.get("ln2_triv", False)
            zs = []
            for k in range(KO):
                z = tmp.tile([P, TT], F16, tag="z", bufs=KO)
                eng = nc.gpsimd if k >= KO - pool_subs else nc.vector
                eng.tensor_tensor(z, x2_sb[t][:, k, :], mu16, ALU.subtract)
                zs.append(z)
            for k in range(KO):
                dst = (
                    h28_sb[t][:, k, :] if k < KF8
                    else h2_sb[t][:, k - KF8, :]
                )
                if ln2_triv:
                    nc.vector.tensor_tensor(dst, zs[k], rstd16, ALU.mult)
                elif k < KF8 and fp8_act:
                    xc = tmp.tile([P, TT], F16, tag="xc")
                    nc.vector.tensor_tensor(xc, zs[k], rstd16, ALU.mult)
                    nc.scalar.activation(
                        dst, xc, AF.Identity,
                        bias=pcol("ln2_b", k), scale=pcol("ln2_g", k),
                    )
                else:
                    nc.vector.tensor_tensor(zs[k], zs[k], rstd16, ALU.mult)
                    nc.vector.tensor_scalar(
                        dst, zs[k],
                        pcol("ln2_g", k), pcol("ln2_b", k),
                        ALU.mult, ALU.add,
                    )

        # ===== software pipeline across the two token tiles ================
        # PE-order: phase1(t0) | stats1(t1) | MLP1(t0) | FFT..LN2(t1) |
        # MLP2(t0) | MLP1(t1) | MLP2(t1).  Tile t1's DVE/ACT-bound LayerNorm
        # chains hide under tile t0's PE-bound MLP matmul stream.
        ps_s = ctx.enter_context(tc.tile_pool(name="ps_s", bufs=1, space="PSUM"))
        ps_q = ctx.enter_context(tc.tile_pool(name="ps_q", bufs=1, space="PSUM"))
        ps_fft = ctx.enter_context(tc.tile_pool(name="ps_fft", bufs=3, space="PSUM"))
        # mlp1's gelu-evac and mlp2's out-evac phases never overlap, so one
        # triple-buffered pool serves both (saves a bank for ps_fft).
        ps_mlp = ctx.enter_context(tc.tile_pool(name="ps_mlp", bufs=3, space="PSUM"))
        ps_out = ps_mlp
        wblk = ctx.enter_context(tc.tile_pool(name="wblk", bufs=1))

        cm_fcos = tc.tile_pool(name="p_fcos", bufs=1, side="right")
        p_fcos = cm_fcos.__enter__()
        cm_xhf = [tc.tile_pool(name=f"p_xhf{t}", bufs=1, side="right")
                  for t in range(NT)]
        # open xhf1 BEFORE xhf0 so the right-side stack pops LIFO:
        # xhf0 (after phase1 t0), then xhf1, then fcos.
        p_xhf = [None, None]
        p_xhf[1] = cm_xhf[1].__enter__()
        p_xhf[0] = cm_xhf[0].__enter__()
        cm_m = [tc.tile_pool(name=f"p_m{t}", bufs=1) for t in range(NT)]

        x16_sb = [p_xhf[t].tile([P, KO, TT], F16, name=f"x16_{t}") for t in range(NT)]
        f_sb = [p_xhf[t].tile([P, KO, TT], F16, name=f"f{t}") for t in range(NT)]
        fcos_sb = p_fcos.tile([P, KO, 5 * P], F16)
        mir_sb = persist.tile([P, 2, P], F16)
        m8_sb = [None, None]

        # tile-0 x16 only; everything else is emitted after the tile-0 stats
        # chain so those matmuls' DMA watermarks cover just these chunks.
        # Single-chunk transfers alternating across two issue queues halve
        # the arrival cadence the startup stats matmuls trickle behind.
        for k in range(KO):
            eng = nc.sync if k % 2 == 0 else nc.scalar
            eng.dma_start(x16_sb[0][:, k : k + 1, :], xT16_r[:, k : k + 1, 0:TT])
        nc.gpsimd.dma_start(par_sb, params[:, :])
        nc.gpsimd.dma_start(mir_sb, mir.rearrange("two q p -> q two p"))

        def load_fcos():
            for k in range(0, KO, 2):
                nc.sync.dma_start(fcos_sb[:, k : k + 2, :], fcos_r[:, k : k + 2, :])

        def load_x16_t1():
            # rides the scalar queue so the sync queue's fcos/x16-t0
            # watermarks (which the tile-0 chain waits on) stay low; must
            # land by ~14us for Pool's tile-1 stats squares.
            for k in range(0, KO, 2):
                nc.scalar.dma_start(
                    x16_sb[1][:, k : k + 2, :], xT16_r[:, k : k + 2, TT : 2 * TT]
                )

        def fft(t, mu16, rstd16, murstd16, mid=None, pre_mirror_cb=None):
            # raw = x16 @ (g*COS); f = rstd*raw - (mu*rstd)*gcs + bfx
            # (LN1 folded into the weights; matmuls depend only on x16).
            # mid[pair] callbacks emit the other tile's LN chain between
            # chunk-pairs so its DVE work starts as early as possible.
            # u_modes (host-detected): per m-chunk, 'none' means gcs==bfx==0
            # there, so the evac is a single rstd-multiply; 'row0' (the DC
            # column of a constant-gamma LN1) additionally patches partition 0
            # with two [1,TT] ops.  'full' keeps the general 3-op chain.
            u_modes = flags.get("u_modes", ("full",) * 5)
            for pair, ms in enumerate([(0, 1), (2, 3), (4,)]):
                if mid is not None and pair in mid:
                    mid[pair]()
                psums = [
                    ps_fft.tile([P, TT], F32, tag="fft", name=f"fft{j}")
                    for j in range(len(ms))
                ]
                for k in range(KO):
                    for j, m in enumerate(ms):
                        nc.tensor.matmul(
                            psums[j],
                            lhsT=fcos_sb[:, k, m * P : (m + 1) * P],
                            rhs=x16_sb[t][:, k, :],
                            start=(k == 0), stop=(k == KO - 1),
                        )
                for j, m in enumerate(ms):
                    if u_modes[m] == "full":
                        q1 = tmp.tile([P, TT], F16, tag="fq")
                        nc.vector.tensor_tensor(q1, psums[j], rstd16, ALU.mult)
                        u = tmp.tile([P, TT], F16, tag="fu")
                        nc.vector.tensor_scalar(
                            u, murstd16,
                            par_sb[:, _GCS_COL + m : _GCS_COL + m + 1],
                            par_sb[:, _BFX_COL + m : _BFX_COL + m + 1],
                            ALU.mult, ALU.subtract,
                        )
                        nc.vector.tensor_tensor(
                            f_sb[t][:, m, :], q1, u, ALU.subtract
                        )
                        continue
                    nc.vector.tensor_tensor(
                        f_sb[t][:, m, :], psums[j], rstd16, ALU.mult
                    )
                    if u_modes[m] == "row0":
                        u0 = tmp.tile([P, TT], F16, tag="fu")
                        nc.vector.tensor_scalar(
                            u0[0:1, :], murstd16[0:1, :],
                            par_sb[0:1, _GCS_COL + m : _GCS_COL + m + 1],
                            par_sb[0:1, _BFX_COL + m : _BFX_COL + m + 1],
                            ALU.mult, ALU.subtract,
                        )
                        nc.vector.tensor_tensor(
                            f_sb[t][0:1, m, :], f_sb[t][0:1, m, :],
                            u0[0:1, :], ALU.subtract,
                        )
            if pre_mirror_cb is not None:
                pre_mirror_cb()
            for m in (5, 6, 7):
                psum_m_ = ps_fft.tile([P, TT], F32, tag="fft", name="fftm")
                nc.tensor.matmul(
                    psum_m_, lhsT=mir_sb[:, 0, :], rhs=f_sb[t][:, 7 - m, :],
                    start=True, stop=False,
                )
                nc.tensor.matmul(
                    psum_m_, lhsT=mir_sb[:, 1, :], rhs=f_sb[t][:, 8 - m, :],
                    start=False, stop=True,
                )
                nc.scalar.activation(f_sb[t][:, m, :], psum_m_, AF.Copy)

        def lnf_stats(t, mu16, rstd16, sq_act=False):
            """stats of f: mean(f) == LN1(x)[0] == g0*(x0-mu)*rstd + b0.
            real-FFT symmetry: f[c] == f[C-c], so sum_c f^2 =
            2*sum_{c<512} f^2 - f0^2 + f512^2 — only chunks 0..3 get
            squared; the two single-row corrections ride a K=1 matmul
            with a 0.5 lhsT (the final ACT scale is 2/C)."""
            psum_s = ps_s.tile([P, TT], F32, tag="ps_s")
            psum_q = ps_q.tile([P, TT], F32, tag="ps_q")
            nc.tensor.matmul(
                psum_s, lhsT=ones_h[0:1, :], rhs=x16_sb[t][0:1, 0, :],
                start=True, stop=True,
            )
            sq_eng = nc.scalar if sq_act else nc.vector
            for k in range(KO // 2):
                sq = tmp.tile([P, TT], F16, tag="sq")
                if sq_act:
                    nc.scalar.activation(sq, f_sb[t][:, k, :], AF.Square)
                else:
                    nc.vector.tensor_mul(sq, f_sb[t][:, k, :], f_sb[t][:, k, :])
                nc.tensor.matmul(
                    psum_q, lhsT=ones_h, rhs=sq,
                    start=(k == 0), stop=False,
                )
            corr = tmp.tile([P, TT], F16, tag="corr")
            c2 = tmp.tile([P, TT], F16, tag="corr2")
            if sq_act:
                nc.scalar.activation(corr[0:1, :], f_sb[t][0:1, 4, :], AF.Square)
                nc.scalar.activation(c2[0:1, :], f_sb[t][0:1, 0, :], AF.Square)
            else:
                nc.vector.tensor_mul(
                    corr[0:1, :], f_sb[t][0:1, 4, :], f_sb[t][0:1, 4, :]
                )
                nc.vector.tensor_mul(
                    c2[0:1, :], f_sb[t][0:1, 0, :], f_sb[t][0:1, 0, :]
                )
            nc.vector.tensor_tensor(corr[0:1, :], corr[0:1, :], c2[0:1, :],
                                    ALU.subtract)
            nc.tensor.matmul(
                psum_q, lhsT=half_h[0:1, :], rhs=corr[0:1, :],
                start=False, stop=True,
            )
            s1 = stat.tile([P, TT], F16, tag="mu")  # becomes muf16
            nc.vector.tensor_tensor(s1, psum_s, mu16, ALU.subtract)
            nc.vector.tensor_tensor(s1, s1, rstd16, ALU.mult)
            if not flags.get("s1_triv", False):
                nc.vector.tensor_scalar(
                    s1, s1,
                    par_sb[:, _G0_COL : _G0_COL + 1],
                    par_sb[:, _B0_COL : _B0_COL + 1],
                    ALU.mult, ALU.add,
                )
            musq = stat.tile([P, TT], F32, tag="musq", bufs=4)
            nc.scalar.activation(musq, s1, AF.Square)
            var = stat.tile([P, TT], F32, tag="var")
            nc.vector.scalar_tensor_tensor(
                var, psum_q, 2.0 / C, musq, ALU.mult, ALU.subtract
            )
            nc.scalar.activation(var, var, AF.Sqrt, bias=eps_sb)
            rstd = stat.tile([P, TT], F32, tag="rstd")
            nc.vector.reciprocal_approx_fast(rstd, var)
            rstdf16 = stat.tile([P, TT], F16, tag="rstd16")
            nc.vector.tensor_copy(rstdf16, rstd)
            return s1, rstdf16

        def lnf_residual(t, muf16, rstdf16, pool_subs=0):
            # the mean-subtraction depends only on muf (ready well before
            # rstdf) — run all 8 up front so only 2 ops/chunk trail rstdf;
            # the last `pool_subs` subtractions go to the idle GpSimd engine.
            fns = []
            for k in range(KO):
                fn = tmp.tile([P, TT], F16, tag="fn", bufs=KO)
                eng = nc.gpsimd if k >= KO - pool_subs else nc.vector
                eng.tensor_tensor(fn, f_sb[t][:, k, :], muf16, ALU.subtract)
                fns.append(fn)
            for k in range(KO):
                nc.vector.tensor_tensor(fns[k], fns[k], rstdf16, ALU.mult)
                nc.vector.affine_then_add(
                    x2_sb[t][:, k, :], fns[k], x16_sb[t][:, k, :],
                    pcol("lnf_g", k), pcol("lnf_b", k),
                )


        def res_pre(t, muf16, rstdf16, mu2_16):
            """lnf-residual, LN2 mean-subtract and the x2 stats squares
            (lnf_triv fast path: mean(LNf(f)) == 0 exactly, so mu2 is tile
            t's mu1 and no x2 sum reduction is needed).  The x2 chunk chain
            is kept to 2 DVE ops/chunk (fn-subs all run up-front; stats
            squares ride ACT; the LN2 subtractions are deferred, with
            chunks 0-1 interleaved early so mlp1's first DoubleRow pair is
            ready as soon as rstd2 lands).  Returns (zs, sq8_tiles)."""
            fns = []
            for k in range(KO):
                fn = tmp.tile([P, TT], F16, tag="fn", bufs=KO)
                nc.vector.tensor_tensor(fn, f_sb[t][:, k, :], muf16, ALU.subtract)
                fns.append(fn)
            zs, sq8s = [None] * KO, []
            lnf_unit = flags.get("lnf_unit", False)
            for k in range(KO):
                nc.vector.tensor_tensor(fns[k], fns[k], rstdf16, ALU.mult)
                if lnf_unit:
                    # gamma==1, beta==0: plain add is ~2x cheaper on DVE
                    # than the affine_then_add custom op (333 vs 722 ns).
                    nc.vector.tensor_tensor(
                        x2_sb[t][:, k, :], fns[k], x16_sb[t][:, k, :], ALU.add
                    )
                else:
                    nc.vector.affine_then_add(
                        x2_sb[t][:, k, :], fns[k], x16_sb[t][:, k, :],
                        pcol("lnf_g", k), pcol("lnf_b", k),
                    )
                if k % 2 == 0:
                    sq8 = tmp.tile(
                        [P, 2, TT], F8, tag="sq8x", bufs=4, name="sq8x"
                    )
                    sq8s.append(sq8)
                nc.scalar.activation(
                    sq8s[-1][:, k % 2, :], x2_sb[t][:, k, :], AF.Square
                )
                if k == 1:
                    for kk in (0, 1):
                        z = tmp.tile([P, TT], F16, tag="z", bufs=KO)
                        nc.vector.tensor_tensor(
                            z, x2_sb[t][:, kk, :], mu2_16, ALU.subtract
                        )
                        zs[kk] = z
            for k in range(2, KO):
                z = tmp.tile([P, TT], F16, tag="z", bufs=KO)
                nc.vector.tensor_tensor(
                    z, x2_sb[t][:, k, :], mu2_16, ALU.subtract
                )
                zs[k] = z
            return zs, sq8s

        def st2_var(sq8s, musq):
            psum_q = ps_q.tile([P, TT], F32, tag="ps_q")
            for j, sq8 in enumerate(sq8s):
                nc.tensor.matmul(
                    psum_q, lhsT=ones_8, rhs=sq8,
                    start=(j == 0), stop=(j == len(sq8s) - 1),
                    perf_mode=DRMODE,
                )
            var = stat.tile([P, TT], F32, tag="var")
            nc.vector.scalar_tensor_tensor(
                var, psum_q, 1.0 / C, musq, ALU.mult, ALU.subtract
            )
            nc.scalar.activation(var, var, AF.Sqrt, bias=eps_sb)
            rstd = stat.tile([P, TT], F32, tag="rstd")
            nc.vector.reciprocal_approx_fast(rstd, var)
            rstd16 = stat.tile([P, TT], F16, tag="rstd16")
            nc.vector.tensor_copy(rstd16, rstd)
            return rstd16

        def ln2_mults(t, zs, rstd16):
            for k in range(KO):
                dst = (
                    h28_sb[t][:, k, :] if k < KF8
                    else h2_sb[t][:, k - KF8, :]
                )
                if flags.get("ln2_triv", False):
                    nc.vector.tensor_tensor(dst, zs[k], rstd16, ALU.mult)
                else:
                    nc.vector.tensor_tensor(zs[k], zs[k], rstd16, ALU.mult)
                    nc.vector.tensor_scalar(
                        dst, zs[k],
                        pcol("ln2_g", k), pcol("ln2_b", k),
                        ALU.mult, ALU.add,
                    )

        w1_pend = {}
        w2_pend = {}

        def issue_w1(h):
            w1blk8 = wblk.tile([P, KF8, P], F8, tag="w1blk8", bufs=8)
            nc.sync.dma_start(w1blk8, w1b8[h])
            w1blk = wblk.tile([P, KO - KF8, P], F16, tag="w1blk", bufs=8)
            nc.sync.dma_start(w1blk, w1b[h])
            w1_pend[h] = (w1blk8, w1blk)

        def issue_w2(c):
            w2blk8 = wblk.tile([P, HF8, P], F8, tag="w2blk8", bufs=3)
            nc.sync.dma_start(w2blk8, w2b8[c])
            w2_pend[c] = w2blk8

        def mlp1(t, h_order=None, keep_last=0, mid=None):
            for bi, h in enumerate(h_order if h_order is not None else range(HO)):
                if mid is not None and bi in mid:
                    mid[bi]()
                if h in w1_pend:
                    w1blk8, w1blk = w1_pend.pop(h)
                else:
                    issue_w1(h)
                    w1blk8, w1blk = w1_pend.pop(h)
                psum_m = ps_mlp.tile([P, TT], F32, tag="mlp1")
                for i in range(A8):
                    nc.tensor.matmul(
                        psum_m,
                        lhsT=w1blk8[:, 2 * i : 2 * i + 2, :],
                        rhs=h28_sb[t][:, 2 * i : 2 * i + 2, :],
                        start=(i == 0), stop=False,
                        perf_mode=DRMODE,
                    )
                for k in range(KO - KF8):
                    nc.tensor.matmul(
                        psum_m, lhsT=w1blk[:, k, :], rhs=h2_sb[t][:, k, :],
                        start=False, stop=(k == KO - KF8 - 1),
                    )
                bias = par_sb[:, _B1_COL + h : _B1_COL + h + 1]
                nc.scalar.activation(
                    m8_sb[t][:, h, :], psum_m, AF.Gelu,
                    bias=bias, scale=1.0 / S1,
                )
                if keep_last and h >= HO - keep_last:
                    # leave the block registered for the next tile's reuse
                    w1_pend[h] = (w1blk8, w1blk)

        def mlp2(t, c_order=None, keep_last=0):
            for c in (c_order if c_order is not None else range(KO)):
                if c in w2_pend:
                    w2blk8 = w2_pend.pop(c)
                else:
                    issue_w2(c)
                    w2blk8 = w2_pend.pop(c)
                psum_o = ps_out.tile([P, TT], F32, tag="mlp1")
                # residual pre-loaded into the psum (scaled to match the
                # S2-scaled weights); all matmuls accumulate on top, and the
                # evacuating activation divides back and adds b2 — no
                # separate DVE residual-add on the output path.
                nc.vector.tensor_scalar_mul(psum_o, x2_sb[t][:, c, :], S2)
                for i in range(B8):
                    nc.tensor.matmul(
                        psum_o,
                        lhsT=w2blk8[:, 2 * i : 2 * i + 2, :],
                        rhs=m8_sb[t][:, 2 * i : 2 * i + 2, :],
                        start=False, stop=(i == B8 - 1),
                        perf_mode=DRMODE,
                    )
                ob = outp.tile([P, TT], F16, tag="ob")
                nc.scalar.activation(
                    ob, psum_o, AF.Identity, bias=pcol("b2", c), scale=1.0 / S2
                )
                nc.sync.dma_start(outT_r[:, c, t * TT : (t + 1) * TT], ob)
                if keep_last and c >= KO - keep_last:
                    w2_pend[c] = w2blk8

        # ---- tile 0 phase 1, tile-1 work woven in as PE filler ----
        lnf_triv = flags.get("lnf_triv", False)
        st1_0 = ln_stats(x16_sb[0], ones_h, ps_s, ps_q)
        load_fcos()
        load_x16_t1()
        mrs0 = stat.tile([P, TT], F16, tag="mrs", name="mrs0", bufs=2)
        nc.vector.tensor_tensor(mrs0, st1_0[0], st1_0[1], ALU.mult)

        stf0 = [None]
        zsq0 = [None]

        def _mid1():
            # lnf stats need only the direct f chunks (0..4) — emitted
            # BEFORE the mirror matmuls so the ACT squares and the muf/rstdf
            # chain start ~4us earlier.
            stf0[0] = lnf_stats(0, *st1_0[:2], sq_act=True)

        fft(0, *st1_0[:2], mrs0, pre_mirror_cb=_mid1)
        # tile-1 stats emitted between the ffts: Pool does its squares (idle
        # engine, loose deadline), keeping ACT free for the tile-0 chain.
        st1_1 = ln_stats(x16_sb[1], ones_h, ps_s, ps_q, sq_eng="pool")
        mrs1 = stat.tile([P, TT], F16, tag="mrs", name="mrs1", bufs=2)
        nc.vector.tensor_tensor(mrs1, st1_1[0], st1_1[1], ALU.mult)

        if lnf_triv:
            def _mid2():
                zsq0[0] = res_pre(0, stf0[0][0], stf0[0][1], st1_0[0])

            fft(1, *st1_1[:2], mrs1, mid={0: _mid2})
            rstd2_0 = st2_var(zsq0[0][1], st1_0[2])
            ln2_mults(0, zsq0[0][0], rstd2_0)
        else:
            def _mid2g():
                lnf_residual(0, *stf0[0], pool_subs=4)

            fft(1, *st1_1[:2], mrs1, mid={1: _mid2g})
            st2_0 = ln_stats(x2_sb[0], ones_h, ps_s, ps_q, sq_act=True)
            ln2_apply(0, *st2_0[:2], pool_subs=4)

        # ---- pipeline ----
        cm_xhf[0].__exit__(None, None, None)
        p_m0 = cm_m[0].__enter__()
        m8_sb[0] = p_m0.tile([P, HF8, TT], F8, name="m8_0")

        # tile-1's LN chain is woven INTO mlp1(0)'s matmul stream (not after
        # it): its PE bits execute between h-blocks and its serial DVE/ACT
        # chain drains ~20us earlier, so the (now much shorter, all-fp8)
        # mlp2(0) stream no longer ends before h28_1 is ready.
        st2_1 = [None]
        stf1 = [None]
        zsq1 = [None]
        rstd2_1 = [None]

        def _m1a():
            stf1[0] = lnf_stats(1, *st1_1[:2])

        if lnf_triv:
            def _m1b():
                zsq1[0] = res_pre(1, stf1[0][0], stf1[0][1], st1_1[0])

            def _m1c():
                rstd2_1[0] = st2_var(zsq1[0][1], st1_1[2])

            def _m1d():
                ln2_mults(1, zsq1[0][0], rstd2_1[0])
        else:
            def _m1b():
                lnf_residual(1, *stf1[0], pool_subs=4)

            def _m1c():
                st2_1[0] = ln_stats(x2_sb[1], ones_h, ps_s, ps_q)

            def _m1d():
                ln2_apply(1, *st2_1[0][:2], fp8_act=False, pool_subs=4)

        mlp1(0, keep_last=8, mid={5: _m1a, 9: _m1b, 16: _m1c, 20: _m1d})

        cm_xhf[1].__exit__(None, None, None)
        cm_fcos.__exit__(None, None, None)
        p_m1 = cm_m[1].__enter__()
        m8_sb[1] = p_m1.tile([P, HF8, TT], F8, name="m8_1")

        mlp2(0, keep_last=3)
        mlp1(1, h_order=list(range(HO - 8, HO)) + list(range(HO - 8)))
        mlp2(1, c_order=[5, 6, 7, 0, 1, 2, 3, 4])

        cm_m[1].__exit__(None, None, None)
        cm_m[0].__exit__(None, None, None)

    nc.compile()
    return nc


_NC_CACHE: dict = {}


def _get_nc(flags):
    key = (
        flags["u_modes"], flags["ln2_triv"], flags["s1_triv"],
        flags["lnf_triv"], flags["lnf_unit"],
    )
    if key not in _NC_CACHE:
        _NC_CACHE[key] = _build_nc(flags)
    return _NC_CACHE[key]


def _host_flags(inputs):
    """Detect trivial LN params so the build can drop dead device work.
    The general path is kept for any input where these don't hold."""
    g1 = np.asarray(inputs["ln1_g"], np.float64)
    b1v = np.asarray(inputs["ln1_b"], np.float64)
    n = np.arange(C, dtype=np.float64)
    cosm = np.cos((np.outer(n, n[: 5 * P]) % C) * (2.0 * np.pi / C))
    gcs = (g1[:, None] * cosm).sum(axis=0)
    bfx = (b1v[:, None] * cosm).sum(axis=0)
    mask = (np.abs(gcs) > 1e-6) | (np.abs(bfx) > 1e-6)
    u_modes = []
    for m in range(5):
        mm = mask[m * P : (m + 1) * P]
        if not mm.any():
            u_modes.append("none")
        elif m == 0 and mm[0] and not mm[1:].any():
            u_modes.append("row0")
        else:
            u_modes.append("full")
    ln2_triv = bool(
        np.allclose(inputs["ln2_g"], 1.0) and np.allclose(inputs["ln2_b"], 0.0)
    )
    s1_triv = bool(abs(g1[0] - 1.0) < 1e-12 and abs(b1v[0]) < 1e-12)
    lg = np.asarray(inputs["lnf_g"], np.float64)
    lb = np.asarray(inputs["lnf_b"], np.float64)
    # constant lnf gain + zero lnf bias make mean(LNf(f)) vanish exactly
    # (sum_c f[c] = C*h[0] = C*mean(f)), so mu2 == mu1 per token.
    lnf_triv = bool(np.ptp(lg) < 1e-12 and np.allclose(lb, 0.0))
    lnf_unit = bool(lnf_triv and abs(lg[0] - 1.0) < 1e-12)
    return {
        "u_modes": tuple(u_modes), "ln2_triv": ln2_triv,
        "s1_triv": s1_triv, "lnf_triv": lnf_triv, "lnf_unit": lnf_unit,
    }


def _pack_params(inputs):
    p = np.zeros((P, _PWIDTH), np.float32)
    for name, col in _PCOLS.items():
        p[:, col : col + 8] = np.asarray(inputs[name], np.float32).reshape(8, P).T
    p[:, _B1_COL : _B1_COL + HO] = (
        np.asarray(inputs["b1"], np.float32).reshape(HO, P).T
    )
    n = np.arange(C, dtype=np.float64)
    cosm = np.cos((np.outer(n, n[: 5 * P]) % C) * (2.0 * np.pi / C))
    g1 = np.asarray(inputs["ln1_g"], np.float64)
    b1v = np.asarray(inputs["ln1_b"], np.float64)
    gcs = (g1[:, None] * cosm).sum(axis=0)          # [640]
    bfx = (b1v[:, None] * cosm).sum(axis=0)         # [640]
    p[:, _GCS_COL : _GCS_COL + 5] = gcs.reshape(5, P).T
    p[:, _BFX_COL : _BFX_COL + 5] = bfx.reshape(5, P).T
    p[:, _G0_COL] = np.float32(g1[0])
    p[:, _B0_COL] = np.float32(b1v[0])
    return p


def _run(inputs, trace=False):
    x = np.asarray(inputs["x"], np.float32)
    B, N, Cc = x.shape
    assert (B * N, Cc) == (N_CORES * TOK, C)
    x2d = x.reshape(B * N, C)

    n = np.arange(C, dtype=np.float64)
    # only the first 5*P output columns are computed directly (f[k] = f[C-k]);
    # LN1's per-channel gain is folded into the DFT matrix, its bias into a
    # per-output-channel additive term (see _pack_params).
    cosm = np.cos((np.outer(n, n[: 5 * P]) % C) * (2.0 * np.pi / C))
    g1 = np.asarray(inputs["ln1_g"], np.float64)
    fcos = (g1[:, None] * cosm).astype(np.float16)

    # weights pre-scaled by S1/S2 (undone in the psum-evacuating activation)
    # in block-contiguous layouts so each SBUF weight block is one clean DMA:
    # w1b[h, kp, ko, hc] = w1[ko*P+kp, h*P+hc]; w2b[c, hp, ho, cc] = w2[ho*P+hp, c*P+cc]
    w1s = np.asarray(inputs["w1"], np.float32) * S1
    w2s = np.asarray(inputs["w2"], np.float32) * S2
    w1bl_f = w1s.reshape(KO, P, HO, P).transpose(2, 1, 0, 3)
    w2bl_f = w2s.reshape(HO, P, KO, P).transpose(2, 1, 0, 3)
    w1bl8 = w1bl_f[:, :, :KF8, :].astype(ml_dtypes.float8_e4m3)
    w1bl = w1bl_f[:, :, KF8:, :].astype(np.float16)
    w2bl8 = w2bl_f[:, :, :HF8, :].astype(ml_dtypes.float8_e4m3)
    # mirror matrices: out[p,t] = f7m[P-p, t] (p>=1);  out[0,t] = f8m[0, t]
    mirm = np.zeros((2, P, P), np.float16)
    for p_ in range(1, P):
        mirm[0, P - p_, p_] = 1.0
    mirm[1, 0, 0] = 1.0
    params = _pack_params(inputs)

    in_maps = []
    for i in range(N_CORES):
        shard = x2d[i * TOK : (i + 1) * TOK, :]
        in_maps.append(
            {
                "xT16": np.ascontiguousarray(shard.T).astype(np.float16),
                "fcos": fcos,
                "w1b8": w1bl8,
                "w1b": w1bl,
                "w2b8": w2bl8,
                "mir": mirm,
                "params": params,
            }
        )

    nc = _get_nc(_host_flags(inputs))
    res = run_bass_kernel_spmd(nc, in_maps, core_ids=list(range(N_CORES)), trace=trace)

    out2d = np.empty((B * N, C), np.float32)
    for i in range(N_CORES):
        out2d[i * TOK : (i + 1) * TOK, :] = res.results[i]["outT"].T
    return out2d.reshape(B, N, C), res


def kernel(**inputs) -> np.ndarray:
    return _run(inputs)[0]



# revision 52
# speedup vs baseline: 1.0362x; 1.0172x over previous
"""Fused Fourier-block kernel for TRN2 (8 NeuronCores, data-parallel).

Reference computation (per token, C=1024, H=4096):
    h  = LN1(x)
    f  = real(FFT_C(h)) = h @ COS            (COS[n,k] = cos(2*pi*n*k/C))
    x2 = x + LNf(f)
    h2 = LN2(x2)
    m  = gelu_exact(h2 @ w1 + b1)
    out = x2 + m @ w2 + b2

Strategy: shard the 4*2048 = 8192 tokens over 8 cores (1024 tokens each).
All device math is done with activations CHANNEL-MAJOR ([channel, token]),
so every matmul consumes weights in their natural [in, out] layout and
chains without any device-side transposes (the host transposes each x shard
on the way in and the output shard on the way out).  LayerNorm reductions
over the channel (partition) dim are done on the TensorEngine as
ones-matmuls whose [128, T] PSUM output broadcasts the per-token sums to
every partition.

Precision/throughput: the DFT matmul stays fp16; half of each MLP
contraction (A8/B8 DoubleRow pairs) runs in fp8e4, which doubles PE
throughput for those chunks (weights pre-scaled by S1/S2 on host, the
inverse scale folded into the psum-evacuating activation; h2/gelu outputs
written in fp8 directly by DVE/ACT).  Sum-of-squares stats reductions also
run as fp8 DoubleRow (x^2 fits e4m3; its ~2.7% noise averages out over
C=1024 terms), and sum f^2 uses the real-FFT mirror symmetry to square
only half the spectrum.  Measured rel err 1.6e-2 vs the 2e-2 gate —
deterministic (fixed inputs), verified on hardware.
"""

from contextlib import ExitStack

import ml_dtypes
import numpy as np

import concourse.bacc as bacc
import concourse.mybir as mybir
import concourse.tile as tile
from concourse.bass_utils import run_bass_kernel_spmd

AF = mybir.ActivationFunctionType
ALU = mybir.AluOpType

P = 128          # SBUF partitions
C = 1024         # channel dim
H = 4096         # MLP hidden dim
KO = C // P      # 8 channel chunks
HO = H // P      # 32 hidden chunks
TOK = 1024       # tokens per core
TT = 512         # token tile (matmul moving dim)
NT = TOK // TT   # 2 token tiles per core
N_CORES = 8
EPS = 1e-5

F32 = mybir.dt.float32
F32R = mybir.dt.float32r
F16 = mybir.dt.float16
F8 = mybir.dt.float8e4
DRMODE = mybir.MatmulPerfMode.DoubleRow

# fp8 fraction of the MLP matmuls (DoubleRow pairs).  A8 of the KO//2=4
# mlp1 contraction chunk-pairs and B8 of the HO//2=16 mlp2 hidden
# block-pairs run in fp8e4; the rest stay fp16.  Weights are pre-scaled
# on host (S1/S2) to keep fp8 off the subnormal range; the inverse scale
# is folded into the psum-evacuating activation.
A8 = 2
B8 = 16
S1 = 16.0
S2 = 32.0
KF8 = 2 * A8          # fp8 channel chunks in mlp1
HF8 = 2 * B8          # fp8 hidden blocks in mlp2

# packed param columns (each [1024] vector becomes [128, 8] partition-major)
_PCOLS = {
    "ln1_g": 0, "ln1_b": 8, "lnf_g": 16, "lnf_b": 24,
    "ln2_g": 32, "ln2_b": 40, "b2": 48,
}
_B1_COL = 56  # b1 occupies cols 56..88
_GCS_COL = 88   # colsum(g*COS) for the 5 direct FFT chunks
_BFX_COL = 93   # (ln1_b @ COS) for the 5 direct FFT chunks
_G0_COL = 98    # ln1_g[0] replicated
_B0_COL = 99    # ln1_b[0] replicated
_PWIDTH = 100


def _build_nc(flags):
    nc = bacc.Bacc()

    xT16 = nc.declare_dram_parameter("xT16", [C, TOK], F16, isOutput=False)
    fcos = nc.declare_dram_parameter("fcos", [C, 5 * P], F16, isOutput=False)
    w1b8 = nc.declare_dram_parameter("w1b8", [HO, P, KF8, P], F8, isOutput=False)
    w1b = nc.declare_dram_parameter("w1b", [HO, P, KO - KF8, P], F16, isOutput=False)
    w2b8 = nc.declare_dram_parameter("w2b8", [KO, P, HF8, P], F8, isOutput=False)
    mir = nc.declare_dram_parameter("mir", [2, P, P], F16, isOutput=False)
    params = nc.declare_dram_parameter("params", [P, _PWIDTH], F32, isOutput=False)
    outT = nc.declare_dram_parameter("outT", [C, TOK], F16, isOutput=True)

    xT16_r = xT16.rearrange("(ko kp) t -> kp ko t", kp=P)
    fcos_r = fcos.rearrange("(ko kp) m -> kp ko m", kp=P)
    outT_r = outT.rearrange("(co cp) t -> cp co t", cp=P)

    with tile.TileContext(nc) as tc, ExitStack() as ctx:
        persist = ctx.enter_context(tc.tile_pool(name="persist", bufs=1))
        tmp = ctx.enter_context(tc.tile_pool(name="tmp", bufs=3))
        stat = ctx.enter_context(tc.tile_pool(name="stat", bufs=3))
        outp = ctx.enter_context(tc.tile_pool(name="outp", bufs=2))

        # ---------- constants ----------
        ones_h = persist.tile([P, P], F16)
        nc.vector.memset(ones_h, 1.0)
        ones_8 = persist.tile([P, 2, P], F8)
        nc.vector.memset(ones_8, 1.0)
        half_h = persist.tile([P, P], F16)
        nc.vector.memset(half_h, 0.5)
        eps_sb = persist.tile([P, 1], F32)
        nc.vector.memset(eps_sb, EPS)

        par_sb = persist.tile([P, _PWIDTH], F32)

        # Touch every activation function once on a dummy [P,1] tile while
        # the engines are idle waiting on the input DMAs: the ACT table
        # loads (~1.3us each) otherwise land lazily on first use, right on
        # the startup stats critical chain.
        act_warm = persist.tile([P, 1], F32)
        for fn in (AF.Copy, AF.Square, AF.Sqrt):
            nc.scalar.activation(act_warm, eps_sb, fn)

        def pcol(name, k):
            c0 = _PCOLS[name] + k
            return par_sb[:, c0 : c0 + 1]

        # activations that live across both phases
        x2_sb = [persist.tile([P, KO, TT], F16, name=f"x2{t}") for t in range(NT)]
        h28_sb = [persist.tile([P, KF8, TT], F8, name=f"h28_{t}") for t in range(NT)]
        h2_sb = [
            persist.tile([P, KO - KF8, TT], F16, name=f"h2{t}") for t in range(NT)
        ]

        def ln_stats(src, ones, ps_s, ps_q, mu_bcast_src=None, sq_act=False,
                     sq_eng=None):
            """src: [P, KO, TT] fp16 tile. Returns (mu16, rstd16) [P, TT] fp16
            broadcast across all partitions. If mu_bcast_src is given (a
            [1, TT] AP already equal to the mean), broadcast it with a single
            K=1 matmul instead of the 8-matmul sum reduction.  The sum-of-
            squares reduction runs as fp8 DoubleRow (x^2 fits e4m3 range and
            its 2.7% noise averages out over C=1024 terms); sq_act moves the
            squaring to the Scalar engine for windows where DVE is the
            critical chain."""
            psum_s = ps_s.tile([P, TT], F32, tag="ps_s")
            psum_q = ps_q.tile([P, TT], F32, tag="ps_q")
            if mu_bcast_src is not None:
                nc.tensor.matmul(
                    psum_s, lhsT=ones[0:1, :], rhs=mu_bcast_src,
                    start=True, stop=True,
                )
            else:
                for k in range(KO):
                    nc.tensor.matmul(
                        psum_s, lhsT=ones, rhs=src[:, k, :],
                        start=(k == 0), stop=(k == KO - 1),
                    )
            if sq_eng is None:
                sq_eng = "act" if sq_act else "dve"
            for j in range(KO // 2):
                sq8 = tmp.tile([P, 2, TT], F8, tag="sq8")
                for jj in range(2):
                    k = 2 * j + jj
                    if sq_eng == "act":
                        nc.scalar.activation(sq8[:, jj, :], src[:, k, :], AF.Square)
                    elif sq_eng == "pool":
                        # Pool is slow (~1.3us/op) but idle during the fill;
                        # tile-1's stats squares have a loose deadline.
                        nc.gpsimd.tensor_mul(
                            sq8[:, jj, :], src[:, k, :], src[:, k, :]
                        )
                    else:
                        nc.vector.tensor_mul(sq8[:, jj, :], src[:, k, :], src[:, k, :])
                nc.tensor.matmul(
                    psum_q, lhsT=ones_8, rhs=sq8,
                    start=(j == 0), stop=(j == KO // 2 - 1),
                    perf_mode=DRMODE,
                )
            mu_scale = 1.0 if mu_bcast_src is not None else 1.0 / C
            mu16 = stat.tile([P, TT], F16, tag="mu")
            nc.scalar.activation(mu16, psum_s, AF.Copy, scale=mu_scale)
            musq = stat.tile([P, TT], F32, tag="musq", bufs=4)
            nc.scalar.activation(musq, psum_s, AF.Square, scale=mu_scale)
            var = stat.tile([P, TT], F32, tag="var")
            nc.vector.scalar_tensor_tensor(
                var, psum_q, 1.0 / C, musq, ALU.mult, ALU.subtract
            )
            nc.scalar.activation(var, var, AF.Sqrt, bias=eps_sb)
            rstd = stat.tile([P, TT], F32, tag="rstd")
            nc.vector.reciprocal_approx_fast(rstd, var)
            rstd16 = stat.tile([P, TT], F16, tag="rstd16")
            nc.vector.tensor_copy(rstd16, rstd)
            return mu16, rstd16, musq

        def ln_apply_chunk(src, mu16, rstd16, gname, bname, dst, k):
            xc = tmp.tile([P, TT], F16, tag="xc")
            nc.vector.tensor_tensor(xc, src[:, k, :], mu16, ALU.subtract)
            nc.vector.tensor_tensor(xc, xc, rstd16, ALU.mult)
            nc.vector.tensor_scalar(
                dst[:, k, :], xc, pcol(gname, k), pcol(bname, k),
                ALU.mult, ALU.add,
            )

        def ln2_apply(t, mu16, rstd16, fp8_act=True, pool_subs=0):
            """h2 = LN2(x2); first KF8 chunks land in fp8.  With ln2_triv
            (gamma==1, beta==0 detected on host) the per-chunk chain is just
            sub + mult, the mult writing the fp8/fp16 tile directly.  The
            mean-subtractions run up front (mu is ready before rstd); the
            last `pool_subs` of them go to the otherwise-idle GpSimd engine.
            """
            ln2_triv = flags.get("ln2_triv", False)
            zs = []
            for k in range(KO):
                z = tmp.tile([P, TT], F16, tag="z", bufs=KO)
                eng = nc.gpsimd if k >= KO - pool_subs else nc.vector
                eng.tensor_tensor(z, x2_sb[t][:, k, :], mu16, ALU.subtract)
                zs.append(z)
            for k in range(KO):
                dst = (
                    h28_sb[t][:, k, :] if k < KF8
                    else h2_sb[t][:, k - KF8, :]
                )
                if ln2_triv:
                    nc.vector.tensor_tensor(dst, zs[k], rstd16, ALU.mult)
                elif k < KF8 and fp8_act:
                    xc = tmp.tile([P, TT], F16, tag="xc")
                    nc.vector.tensor_tensor(xc, zs[k], rstd16, ALU.mult)
                    nc.scalar.activation(
                        dst, xc, AF.Identity,
                        bias=pcol("ln2_b", k), scale=pcol("ln2_g", k),
                    )
                else:
                    nc.vector.tensor_tensor(zs[k], zs[k], rstd16, ALU.mult)
                    nc.vector.tensor_scalar(
                        dst, zs[k],
                        pcol("ln2_g", k), pcol("ln2_b", k),
                        ALU.mult, ALU.add,
                    )

        # ===== software pipeline across the two token tiles ================
        # PE-order: phase1(t0) | stats1(t1) | MLP1(t0) | FFT..LN2(t1) |
        # MLP2(t0) | MLP1(t1) | MLP2(t1).  Tile t1's DVE/ACT-bound LayerNorm
        # chains hide under tile t0's PE-bound MLP matmul stream.
        ps_s = ctx.enter_context(tc.tile_pool(name="ps_s", bufs=1, space="PSUM"))
        ps_q = ctx.enter_context(tc.tile_pool(name="ps_q", bufs=1, space="PSUM"))
        ps_fft = ctx.enter_context(tc.tile_pool(name="ps_fft", bufs=3, space="PSUM"))
        # mlp1's gelu-evac and mlp2's out-evac phases never overlap, so one
        # triple-buffered pool serves both (saves a bank for ps_fft).
        ps_mlp = ctx.enter_context(tc.tile_pool(name="ps_mlp", bufs=3, space="PSUM"))
        ps_out = ps_mlp
        wblk = ctx.enter_context(tc.tile_pool(name="wblk", bufs=1))

        cm_fcos = tc.tile_pool(name="p_fcos", bufs=1, side="right")
        p_fcos = cm_fcos.__enter__()
        cm_xhf = [tc.tile_pool(name=f"p_xhf{t}", bufs=1, side="right")
                  for t in range(NT)]
        # open xhf1 BEFORE xhf0 so the right-side stack pops LIFO:
        # xhf0 (after phase1 t0), then xhf1, then fcos.
        p_xhf = [None, None]
        p_xhf[1] = cm_xhf[1].__enter__()
        p_xhf[0] = cm_xhf[0].__enter__()
        cm_m = [tc.tile_pool(name=f"p_m{t}", bufs=1) for t in range(NT)]

        x16_sb = [p_xhf[t].tile([P, KO, TT], F16, name=f"x16_{t}") for t in range(NT)]
        f_sb = [p_xhf[t].tile([P, KO, TT], F16, name=f"f{t}") for t in range(NT)]
        fcos_sb = p_fcos.tile([P, KO, 5 * P], F16)
        mir_sb = persist.tile([P, 2, P], F16)
        m8_sb = [None, None]

        # tile-0 x16 only; everything else is emitted after the tile-0 stats
        # chain so those matmuls' DMA watermarks cover just these chunks.
        # Single-chunk transfers alternating across two issue queues halve
        # the arrival cadence the startup stats matmuls trickle behind.
        for k in range(KO):
            eng = nc.sync if k % 2 == 0 else nc.scalar
            eng.dma_start(x16_sb[0][:, k : k + 1, :], xT16_r[:, k : k + 1, 0:TT])
        nc.gpsimd.dma_start(par_sb, params[:, :])
        nc.gpsimd.dma_start(mir_sb, mir.rearrange("two q p -> q two p"))

        def load_fcos():
            for k in range(0, KO, 2):
                nc.sync.dma_start(fcos_sb[:, k : k + 2, :], fcos_r[:, k : k + 2, :])

        def load_x16_t1():
            # rides the scalar queue so the sync queue's fcos/x16-t0
            # watermarks (which the tile-0 chain waits on) stay low; must
            # land by ~14us for Pool's tile-1 stats squares.
            for k in range(0, KO, 2):
                nc.scalar.dma_start(
                    x16_sb[1][:, k : k + 2, :], xT16_r[:, k : k + 2, TT : 2 * TT]
                )

        def fft(t, mu16, rstd16, murstd16, mid=None, pre_mirror_cb=None):
            # raw = x16 @ (g*COS); f = rstd*raw - (mu*rstd)*gcs + bfx
            # (LN1 folded into the weights; matmuls depend only on x16).
            # mid[pair] callbacks emit the other tile's LN chain between
            # chunk-pairs so its DVE work starts as early as possible.
            # u_modes (host-detected): per m-chunk, 'none' means gcs==bfx==0
            # there, so the evac is a single rstd-multiply; 'row0' (the DC
            # column of a constant-gamma LN1) additionally patches partition 0
            # with two [1,TT] ops.  'full' keeps the general 3-op chain.
            u_modes = flags.get("u_modes", ("full",) * 5)
            for pair, ms in enumerate([(0, 1), (2, 3), (4,)]):
                if mid is not None and pair in mid:
                    mid[pair]()
                psums = [
                    ps_fft.tile([P, TT], F32, tag="fft", name=f"fft{j}")
                    for j in range(len(ms))
                ]
                for k in range(KO):
                    for j, m in enumerate(ms):
                        nc.tensor.matmul(
                            psums[j],
                            lhsT=fcos_sb[:, k, m * P : (m + 1) * P],
                            rhs=x16_sb[t][:, k, :],
                            start=(k == 0), stop=(k == KO - 1),
                        )
                for j, m in enumerate(ms):
                    if u_modes[m] == "full":
                        q1 = tmp.tile([P, TT], F16, tag="fq")
                        nc.vector.tensor_tensor(q1, psums[j], rstd16, ALU.mult)
                        u = tmp.tile([P, TT], F16, tag="fu")
                        nc.vector.tensor_scalar(
                            u, murstd16,
                            par_sb[:, _GCS_COL + m : _GCS_COL + m + 1],
                            par_sb[:, _BFX_COL + m : _BFX_COL + m + 1],
                            ALU.mult, ALU.subtract,
                        )
                        nc.vector.tensor_tensor(
                            f_sb[t][:, m, :], q1, u, ALU.subtract
                        )
                        continue
                    nc.vector.tensor_tensor(
                        f_sb[t][:, m, :], psums[j], rstd16, ALU.mult
                    )
                    if u_modes[m] == "row0":
                        u0 = tmp.tile([P, TT], F16, tag="fu")
                        nc.vector.tensor_scalar(
                            u0[0:1, :], murstd16[0:1, :],
                            par_sb[0:1, _GCS_COL + m : _GCS_COL + m + 1],
                            par_sb[0:1, _BFX_COL + m : _BFX_COL + m + 1],
                            ALU.mult, ALU.subtract,
                        )
                        nc.vector.tensor_tensor(
                            f_sb[t][0:1, m, :], f_sb[t][0:1, m, :],
                            u0[0:1, :], ALU.subtract,
                        )
            if pre_mirror_cb is not None:
                pre_mirror_cb()
            for m in (5, 6, 7):
                psum_m_ = ps_fft.tile([P, TT], F32, tag="fft", name="fftm")
                nc.tensor.matmul(
                    psum_m_, lhsT=mir_sb[:, 0, :], rhs=f_sb[t][:, 7 - m, :],
                    start=True, stop=False,
                )
                nc.tensor.matmul(
                    psum_m_, lhsT=mir_sb[:, 1, :], rhs=f_sb[t][:, 8 - m, :],
                    start=False, stop=True,
                )
                nc.scalar.activation(f_sb[t][:, m, :], psum_m_, AF.Copy)

        def lnf_stats(t, mu16, rstd16, sq_act=False):
            """stats of f: mean(f) == LN1(x)[0] == g0*(x0-mu)*rstd + b0.
            real-FFT symmetry: f[c] == f[C-c], so sum_c f^2 =
            2*sum_{c<512} f^2 - f0^2 + f512^2 — only chunks 0..3 get
            squared; the two single-row corrections ride a K=1 matmul
            with a 0.5 lhsT (the final ACT scale is 2/C)."""
            psum_s = ps_s.tile([P, TT], F32, tag="ps_s")
            psum_q = ps_q.tile([P, TT], F32, tag="ps_q")
            nc.tensor.matmul(
                psum_s, lhsT=ones_h[0:1, :], rhs=x16_sb[t][0:1, 0, :],
                start=True, stop=True,
            )
            sq_eng = nc.scalar if sq_act else nc.vector
            for k in range(KO // 2):
                sq = tmp.tile([P, TT], F16, tag="sq")
                if sq_act:
                    nc.scalar.activation(sq, f_sb[t][:, k, :], AF.Square)
                else:
                    nc.vector.tensor_mul(sq, f_sb[t][:, k, :], f_sb[t][:, k, :])
                nc.tensor.matmul(
                    psum_q, lhsT=ones_h, rhs=sq,
                    start=(k == 0), stop=False,
                )
            corr = tmp.tile([P, TT], F16, tag="corr")
            c2 = tmp.tile([P, TT], F16, tag="corr2")
            if sq_act:
                nc.scalar.activation(corr[0:1, :], f_sb[t][0:1, 4, :], AF.Square)
                nc.scalar.activation(c2[0:1, :], f_sb[t][0:1, 0, :], AF.Square)
            else:
                nc.vector.tensor_mul(
                    corr[0:1, :], f_sb[t][0:1, 4, :], f_sb[t][0:1, 4, :]
                )
                nc.vector.tensor_mul(
                    c2[0:1, :], f_sb[t][0:1, 0, :], f_sb[t][0:1, 0, :]
                )
            nc.vector.tensor_tensor(corr[0:1, :], corr[0:1, :], c2[0:1, :],
                                    ALU.subtract)
            nc.tensor.matmul(
                psum_q, lhsT=half_h[0:1, :], rhs=corr[0:1, :],
                start=False, stop=True,
            )
            s1 = stat.tile([P, TT], F16, tag="mu")  # becomes muf16
            nc.vector.tensor_tensor(s1, psum_s, mu16, ALU.subtract)
            nc.vector.tensor_tensor(s1, s1, rstd16, ALU.mult)
            if not flags.get("s1_triv", False):
                nc.vector.tensor_scalar(
                    s1, s1,
                    par_sb[:, _G0_COL : _G0_COL + 1],
                    par_sb[:, _B0_COL : _B0_COL + 1],
                    ALU.mult, ALU.add,
                )
            musq = stat.tile([P, TT], F32, tag="musq", bufs=4)
            nc.vector.tensor_mul(musq, s1, s1)
            var = stat.tile([P, TT], F32, tag="var")
            nc.vector.scalar_tensor_tensor(
                var, psum_q, 2.0 / C, musq, ALU.mult, ALU.subtract
            )
            nc.scalar.activation(var, var, AF.Sqrt, bias=eps_sb)
            rstd = stat.tile([P, TT], F32, tag="rstd")
            nc.vector.reciprocal_approx_fast(rstd, var)
            rstdf16 = stat.tile([P, TT], F16, tag="rstd16")
            nc.vector.tensor_copy(rstdf16, rstd)
            return s1, rstdf16

        def lnf_residual(t, muf16, rstdf16, pool_subs=0):
            # the mean-subtraction depends only on muf (ready well before
            # rstdf) — run all 8 up front so only 2 ops/chunk trail rstdf;
            # the last `pool_subs` subtractions go to the idle GpSimd engine.
            fns = []
            for k in range(KO):
                fn = tmp.tile([P, TT], F16, tag="fn", bufs=KO)
                eng = nc.gpsimd if k >= KO - pool_subs else nc.vector
                eng.tensor_tensor(fn, f_sb[t][:, k, :], muf16, ALU.subtract)
                fns.append(fn)
            for k in range(KO):
                nc.vector.tensor_tensor(fns[k], fns[k], rstdf16, ALU.mult)
                nc.vector.affine_then_add(
                    x2_sb[t][:, k, :], fns[k], x16_sb[t][:, k, :],
                    pcol("lnf_g", k), pcol("lnf_b", k),
                )


        def res_pre(t, muf16, rstdf16, mu2_16, sq_eng="act"):
            """lnf-residual, LN2 mean-subtract and the x2 stats squares
            (lnf_triv fast path: mean(LNf(f)) == 0 exactly, so mu2 is tile
            t's mu1 and no x2 sum reduction is needed).  The x2 chunk chain
            is kept to 2 DVE ops/chunk (fn-subs all run up-front; stats
            squares ride ACT; the LN2 subtractions are deferred, with
            chunks 0-1 interleaved early so mlp1's first DoubleRow pair is
            ready as soon as rstd2 lands).  Returns (zs, sq8_tiles)."""
            fns = []
            for k in range(KO):
                fn = tmp.tile([P, TT], F16, tag="fn", bufs=KO)
                nc.vector.tensor_tensor(fn, f_sb[t][:, k, :], muf16, ALU.subtract)
                fns.append(fn)
            zs, sq8s = [None] * KO, []
            lnf_unit = flags.get("lnf_unit", False)
            for k in range(KO):
                nc.vector.tensor_tensor(fns[k], fns[k], rstdf16, ALU.mult)
                if lnf_unit:
                    # gamma==1, beta==0: plain add is ~2x cheaper on DVE
                    # than the affine_then_add custom op (333 vs 722 ns).
                    nc.vector.tensor_tensor(
                        x2_sb[t][:, k, :], fns[k], x16_sb[t][:, k, :], ALU.add
                    )
                else:
                    nc.vector.affine_then_add(
                        x2_sb[t][:, k, :], fns[k], x16_sb[t][:, k, :],
                        pcol("lnf_g", k), pcol("lnf_b", k),
                    )
                if k % 2 == 0:
                    sq8 = tmp.tile(
                        [P, 2, TT], F8, tag="sq8x", bufs=4, name="sq8x"
                    )
                    sq8s.append(sq8)
                if sq_eng == "act":
                    nc.scalar.activation(
                        sq8s[-1][:, k % 2, :], x2_sb[t][:, k, :], AF.Square
                    )
                else:
                    # tile-1 squares ride Pool: an ACT Square between the
                    # gelu evacuations would thrash the activation table
                    # (~1.3us reload per function switch).
                    nc.gpsimd.tensor_mul(
                        sq8s[-1][:, k % 2, :],
                        x2_sb[t][:, k, :], x2_sb[t][:, k, :],
                    )
                if k == 1:
                    for kk in (0, 1):
                        z = tmp.tile([P, TT], F16, tag="z", bufs=KO)
                        nc.vector.tensor_tensor(
                            z, x2_sb[t][:, kk, :], mu2_16, ALU.subtract
                        )
                        zs[kk] = z
            for k in range(2, KO):
                z = tmp.tile([P, TT], F16, tag="z", bufs=KO)
                nc.vector.tensor_tensor(
                    z, x2_sb[t][:, k, :], mu2_16, ALU.subtract
                )
                zs[k] = z
            return zs, sq8s

        def st2_var(sq8s, musq):
            psum_q = ps_q.tile([P, TT], F32, tag="ps_q")
            for j, sq8 in enumerate(sq8s):
                nc.tensor.matmul(
                    psum_q, lhsT=ones_8, rhs=sq8,
                    start=(j == 0), stop=(j == len(sq8s) - 1),
                    perf_mode=DRMODE,
                )
            var = stat.tile([P, TT], F32, tag="var")
            nc.vector.scalar_tensor_tensor(
                var, psum_q, 1.0 / C, musq, ALU.mult, ALU.subtract
            )
            nc.scalar.activation(var, var, AF.Sqrt, bias=eps_sb)
            rstd = stat.tile([P, TT], F32, tag="rstd")
            nc.vector.reciprocal_approx_fast(rstd, var)
            rstd16 = stat.tile([P, TT], F16, tag="rstd16")
            nc.vector.tensor_copy(rstd16, rstd)
            return rstd16

        def ln2_mults(t, zs, rstd16):
            for k in range(KO):
                dst = (
                    h28_sb[t][:, k, :] if k < KF8
                    else h2_sb[t][:, k - KF8, :]
                )
                if flags.get("ln2_triv", False):
                    nc.vector.tensor_tensor(dst, zs[k], rstd16, ALU.mult)
                else:
                    nc.vector.tensor_tensor(zs[k], zs[k], rstd16, ALU.mult)
                    nc.vector.tensor_scalar(
                        dst, zs[k],
                        pcol("ln2_g", k), pcol("ln2_b", k),
                        ALU.mult, ALU.add,
                    )

        w1_pend = {}
        w2_pend = {}

        def issue_w1(h):
            w1blk8 = wblk.tile([P, KF8, P], F8, tag="w1blk8", bufs=8)
            nc.sync.dma_start(w1blk8, w1b8[h])
            w1blk = wblk.tile([P, KO - KF8, P], F16, tag="w1blk", bufs=8)
            nc.sync.dma_start(w1blk, w1b[h])
            w1_pend[h] = (w1blk8, w1blk)

        def issue_w2(c):
            w2blk8 = wblk.tile([P, HF8, P], F8, tag="w2blk8", bufs=3)
            nc.sync.dma_start(w2blk8, w2b8[c])
            w2_pend[c] = w2blk8

        def mlp1(t, h_order=None, keep_last=0, mid=None):
            for bi, h in enumerate(h_order if h_order is not None else range(HO)):
                if mid is not None and bi in mid:
                    mid[bi]()
                if h in w1_pend:
                    w1blk8, w1blk = w1_pend.pop(h)
                else:
                    issue_w1(h)
                    w1blk8, w1blk = w1_pend.pop(h)
                psum_m = ps_mlp.tile([P, TT], F32, tag="mlp1")
                for i in range(A8):
                    nc.tensor.matmul(
                        psum_m,
                        lhsT=w1blk8[:, 2 * i : 2 * i + 2, :],
                        rhs=h28_sb[t][:, 2 * i : 2 * i + 2, :],
                        start=(i == 0), stop=False,
                        perf_mode=DRMODE,
                    )
                for k in range(KO - KF8):
                    nc.tensor.matmul(
                        psum_m, lhsT=w1blk[:, k, :], rhs=h2_sb[t][:, k, :],
                        start=False, stop=(k == KO - KF8 - 1),
                    )
                bias = par_sb[:, _B1_COL + h : _B1_COL + h + 1]
                nc.scalar.activation(
                    m8_sb[t][:, h, :], psum_m, AF.Gelu,
                    bias=bias, scale=1.0 / S1,
                )
                if keep_last and h >= HO - keep_last:
                    # leave the block registered for the next tile's reuse
                    w1_pend[h] = (w1blk8, w1blk)

        def mlp2(t, c_order=None, keep_last=0):
            for c in (c_order if c_order is not None else range(KO)):
                if c in w2_pend:
                    w2blk8 = w2_pend.pop(c)
                else:
                    issue_w2(c)
                    w2blk8 = w2_pend.pop(c)
                psum_o = ps_out.tile([P, TT], F32, tag="mlp1")
                # residual pre-loaded into the psum (scaled to match the
                # S2-scaled weights); all matmuls accumulate on top, and the
                # evacuating activation divides back and adds b2 — no
                # separate DVE residual-add on the output path.
                nc.vector.tensor_scalar_mul(psum_o, x2_sb[t][:, c, :], S2)
                for i in range(B8):
                    nc.tensor.matmul(
                        psum_o,
                        lhsT=w2blk8[:, 2 * i : 2 * i + 2, :],
                        rhs=m8_sb[t][:, 2 * i : 2 * i + 2, :],
                        start=False, stop=(i == B8 - 1),
                        perf_mode=DRMODE,
                    )
                ob = outp.tile([P, TT], F16, tag="ob")
                nc.scalar.activation(
                    ob, psum_o, AF.Identity, bias=pcol("b2", c), scale=1.0 / S2
                )
                nc.sync.dma_start(outT_r[:, c, t * TT : (t + 1) * TT], ob)
                if keep_last and c >= KO - keep_last:
                    w2_pend[c] = w2blk8

        # ---- tile 0 phase 1, tile-1 work woven in as PE filler ----
        lnf_triv = flags.get("lnf_triv", False)
        st1_0 = ln_stats(x16_sb[0], ones_h, ps_s, ps_q)
        load_fcos()
        load_x16_t1()
        mrs0 = stat.tile([P, TT], F16, tag="mrs", name="mrs0", bufs=2)
        nc.vector.tensor_tensor(mrs0, st1_0[0], st1_0[1], ALU.mult)

        stf0 = [None]
        zsq0 = [None]

        def _mid1():
            # lnf stats need only the direct f chunks (0..4) — emitted
            # BEFORE the mirror matmuls so the ACT squares and the muf/rstdf
            # chain start ~4us earlier.
            stf0[0] = lnf_stats(0, *st1_0[:2], sq_act=True)

        fft(0, *st1_0[:2], mrs0, pre_mirror_cb=_mid1)
        # tile-1 stats emitted between the ffts: Pool does its squares (idle
        # engine, loose deadline), keeping ACT free for the tile-0 chain.
        st1_1 = ln_stats(x16_sb[1], ones_h, ps_s, ps_q, sq_eng="pool")
        mrs1 = stat.tile([P, TT], F16, tag="mrs", name="mrs1", bufs=2)
        nc.vector.tensor_tensor(mrs1, st1_1[0], st1_1[1], ALU.mult)

        if lnf_triv:
            def _mid2():
                zsq0[0] = res_pre(0, stf0[0][0], stf0[0][1], st1_0[0])

            fft(1, *st1_1[:2], mrs1, mid={0: _mid2})
            rstd2_0 = st2_var(zsq0[0][1], st1_0[2])
            ln2_mults(0, zsq0[0][0], rstd2_0)
        else:
            def _mid2g():
                lnf_residual(0, *stf0[0], pool_subs=4)

            fft(1, *st1_1[:2], mrs1, mid={1: _mid2g})
            st2_0 = ln_stats(x2_sb[0], ones_h, ps_s, ps_q, sq_act=True)
            ln2_apply(0, *st2_0[:2], pool_subs=4)

        # ---- pipeline ----
        cm_xhf[0].__exit__(None, None, None)
        p_m0 = cm_m[0].__enter__()
        m8_sb[0] = p_m0.tile([P, HF8, TT], F8, name="m8_0")

        # tile-1's LN chain is woven INTO mlp1(0)'s matmul stream (not after
        # it): its PE bits execute between h-blocks and its serial DVE/ACT
        # chain drains ~20us earlier, so the (now much shorter, all-fp8)
        # mlp2(0) stream no longer ends before h28_1 is ready.
        st2_1 = [None]
        stf1 = [None]
        zsq1 = [None]
        rstd2_1 = [None]

        def _m1a():
            stf1[0] = lnf_stats(1, *st1_1[:2])

        if lnf_triv:
            def _m1b():
                zsq1[0] = res_pre(
                    1, stf1[0][0], stf1[0][1], st1_1[0], sq_eng="pool"
                )

            def _m1c():
                rstd2_1[0] = st2_var(zsq1[0][1], st1_1[2])

            def _m1d():
                ln2_mults(1, zsq1[0][0], rstd2_1[0])
        else:
            def _m1b():
                lnf_residual(1, *stf1[0], pool_subs=4)

            def _m1c():
                st2_1[0] = ln_stats(x2_sb[1], ones_h, ps_s, ps_q)

            def _m1d():
                ln2_apply(1, *st2_1[0][:2], fp8_act=False, pool_subs=4)

        mlp1(0, keep_last=8, mid={5: _m1a, 9: _m1b, 16: _m1c, 20: _m1d})

        cm_xhf[1].__exit__(None, None, None)
        cm_fcos.__exit__(None, None, None)
        p_m1 = cm_m[1].__enter__()
        m8_sb[1] = p_m1.tile([P, HF8, TT], F8, name="m8_1")

        mlp2(0, keep_last=3)
        mlp1(1, h_order=list(range(HO - 8, HO)) + list(range(HO - 8)))
        mlp2(1, c_order=[5, 6, 7, 0, 1, 2, 3, 4])

        cm_m[1].__exit__(None, None, None)
        cm_m[0].__exit__(None, None, None)

    nc.compile()
    return nc


_NC_CACHE: dict = {}


def _get_nc(flags):
    key = (
        flags["u_modes"], flags["ln2_triv"], flags["s1_triv"],
        flags["lnf_triv"], flags["lnf_unit"],
    )
    if key not in _NC_CACHE:
        _NC_CACHE[key] = _build_nc(flags)
    return _NC_CACHE[key]


def _host_flags(inputs):
    """Detect trivial LN params so the build can drop dead device work.
    The general path is kept for any input where these don't hold."""
    g1 = np.asarray(inputs["ln1_g"], np.float64)
    b1v = np.asarray(inputs["ln1_b"], np.float64)
    n = np.arange(C, dtype=np.float64)
    cosm = np.cos((np.outer(n, n[: 5 * P]) % C) * (2.0 * np.pi / C))
    gcs = (g1[:, None] * cosm).sum(axis=0)
    bfx = (b1v[:, None] * cosm).sum(axis=0)
    mask = (np.abs(gcs) > 1e-6) | (np.abs(bfx) > 1e-6)
    u_modes = []
    for m in range(5):
        mm = mask[m * P : (m + 1) * P]
        if not mm.any():
            u_modes.append("none")
        elif m == 0 and mm[0] and not mm[1:].any():
            u_modes.append("row0")
        else:
            u_modes.append("full")
    ln2_triv = bool(
        np.allclose(inputs["ln2_g"], 1.0) and np.allclose(inputs["ln2_b"], 0.0)
    )
    s1_triv = bool(abs(g1[0] - 1.0) < 1e-12 and abs(b1v[0]) < 1e-12)
    lg = np.asarray(inputs["lnf_g"], np.float64)
    lb = np.asarray(inputs["lnf_b"], np.float64)
    # constant lnf gain + zero lnf bias make mean(LNf(f)) vanish exactly
    # (sum_c f[c] = C*h[0] = C*mean(f)), so mu2 == mu1 per token.
    lnf_triv = bool(np.ptp(lg) < 1e-12 and np.allclose(lb, 0.0))
    lnf_unit = bool(lnf_triv and abs(lg[0] - 1.0) < 1e-12)
    return {
        "u_modes": tuple(u_modes), "ln2_triv": ln2_triv,
        "s1_triv": s1_triv, "lnf_triv": lnf_triv, "lnf_unit": lnf_unit,
    }


def _pack_params(inputs):
    p = np.zeros((P, _PWIDTH), np.float32)
    for name, col in _PCOLS.items():
        p[:, col : col + 8] = np.asarray(inputs[name], np.float32).reshape(8, P).T
    p[:, _B1_COL : _B1_COL + HO] = (
        np.asarray(inputs["b1"], np.float32).reshape(HO, P).T
    )
    n = np.arange(C, dtype=np.float64)
    cosm = np.cos((np.outer(n, n[: 5 * P]) % C) * (2.0 * np.pi / C))
    g1 = np.asarray(inputs["ln1_g"], np.float64)
    b1v = np.asarray(inputs["ln1_b"], np.float64)
    gcs = (g1[:, None] * cosm).sum(axis=0)          # [640]
    bfx = (b1v[:, None] * cosm).sum(axis=0)         # [640]
    p[:, _GCS_COL : _GCS_COL + 5] = gcs.reshape(5, P).T
    p[:, _BFX_COL : _BFX_COL + 5] = bfx.reshape(5, P).T
    p[:, _G0_COL] = np.float32(g1[0])
    p[:, _B0_COL] = np.float32(b1v[0])
    return p


def _run(inputs, trace=False):
    x = np.asarray(inputs["x"], np.float32)
    B, N, Cc = x.shape
    assert (B * N, Cc) == (N_CORES * TOK, C)
    x2d = x.reshape(B * N, C)

    n = np.arange(C, dtype=np.float64)
    # only the first 5*P output columns are computed directly (f[k] = f[C-k]);
    # LN1's per-channel gain is folded into the DFT matrix, its bias into a
    # per-output-channel additive term (see _pack_params).
    cosm = np.cos((np.outer(n, n[: 5 * P]) % C) * (2.0 * np.pi / C))
    g1 = np.asarray(inputs["ln1_g"], np.float64)
    fcos = (g1[:, None] * cosm).astype(np.float16)

    # weights pre-scaled by S1/S2 (undone in the psum-evacuating activation)
    # in block-contiguous layouts so each SBUF weight block is one clean DMA:
    # w1b[h, kp, ko, hc] = w1[ko*P+kp, h*P+hc]; w2b[c, hp, ho, cc] = w2[ho*P+hp, c*P+cc]
    w1s = np.asarray(inputs["w1"], np.float32) * S1
    w2s = np.asarray(inputs["w2"], np.float32) * S2
    w1bl_f = w1s.reshape(KO, P, HO, P).transpose(2, 1, 0, 3)
    w2bl_f = w2s.reshape(HO, P, KO, P).transpose(2, 1, 0, 3)
    w1bl8 = w1bl_f[:, :, :KF8, :].astype(ml_dtypes.float8_e4m3)
    w1bl = w1bl_f[:, :, KF8:, :].astype(np.float16)
    w2bl8 = w2bl_f[:, :, :HF8, :].astype(ml_dtypes.float8_e4m3)
    # mirror matrices: out[p,t] = f7m[P-p, t] (p>=1);  out[0,t] = f8m[0, t]
    mirm = np.zeros((2, P, P), np.float16)
    for p_ in range(1, P):
        mirm[0, P - p_, p_] = 1.0
    mirm[1, 0, 0] = 1.0
    params = _pack_params(inputs)

    in_maps = []
    for i in range(N_CORES):
        shard = x2d[i * TOK : (i + 1) * TOK, :]
        in_maps.append(
            {
                "xT16": np.ascontiguousarray(shard.T).astype(np.float16),
                "fcos": fcos,
                "w1b8": w1bl8,
                "w1b": w1bl,
                "w2b8": w2bl8,
                "mir": mirm,
                "params": params,
            }
        )

    nc = _get_nc(_host_flags(inputs))
    res = run_bass_kernel_spmd(nc, in_maps, core_ids=list(range(N_CORES)), trace=trace)

    out2d = np.empty((B * N, C), np.float32)
    for i in range(N_CORES):
        out2d[i * TOK : (i + 1) * TOK, :] = res.results[i]["outT"].T
    return out2d.reshape(B, N, C), res


def kernel(**inputs) -> np.ndarray:
    return _run(inputs)[0]

